# revision 2
# baseline (speedup 1.0000x reference)
"""Trainium2 Bass kernel for nn_CausalMixer (QMIX-style causal mixer).

Data-parallel across 8 NeuronCores: batch dim sharded round-robin
(core m gets batches m, m+8, m+16, ...), hypernet weights replicated.

Per-core layout (R = 1024 rows = 16 batches x 64 timesteps):
  - stage-1 "transposed" GEMMs: out[feat, rows] = Wcat.T-chunks @ states.T,
    evicted with fused per-partition bias+ReLU on ScalarE.
  - stage-2 row-major GEMMs: the relu'd z tiles [feat, rows] serve directly
    as lhsT, producing per-row hypernet weights [rows, feat]; bias is
    preloaded into PSUM with a K=1 ones-matmul.
  - gather (qvals[cr]) via 10x {is_equal mask + copy_predicated} on DVE.
  - the onehot quirk (batch row b==v gets +delta) is handled as a rank-1
    correction on chunk 0 only (host orders the diag batches first).
"""

import sys

for _p in ("/root/.axon_site", "/root/.axon_site/_ro/trn_rl_repo",
           "/root/.axon_site/_ro/pypackages", "/opt/trn_rl_repo"):
    if _p not in sys.path:
        sys.path.append(_p)

import numpy as np
import ml_dtypes
from contextlib import ExitStack

import concourse.bass as bass
import concourse.bacc as bacc_mod
import concourse.tile as tile
import concourse.mybir as mybir
from concourse.bass import broadcast_tensor_aps
from concourse.bass_utils import run_bass_kernel_spmd

BF = ml_dtypes.bfloat16
DT = mybir.dt.bfloat16
F32 = mybir.dt.float32
U16 = mybir.dt.uint16
OP = mybir.AluOpType
AF = mybir.ActivationFunctionType
AX = mybir.AxisListType

NCORES = 8
B, T, NA, NV, K, SD, H, E = 128, 64, 10, 16, 4, 512, 256, 32
R = 16 * T            # rows per core = 1024
C = R // 128          # row chunks per core = 8
NVK = NV * K          # 64
JW = NVK + K          # 68 gather cols (64 + 4 diag)
SMALL = NA + 1 + 1 + E + E   # 76: w01 | b01 | b00 | b1 | b2_l1
W1COL = (NV + 1) * E  # 544

_cache = {}


def _build_nc():
    nc = bacc_mod.Bacc("TRN2", target_bir_lowering=False, debug=False)

    s_t_d = nc.dram_tensor("s_t", [SD, R], DT, kind="ExternalInput")
    wcat_d = nc.dram_tensor("wcat", [SD, 768], DT, kind="ExternalInput")
    wsmall_d = nc.dram_tensor("wsmall", [SD, SMALL], DT, kind="ExternalInput")
    w1l2_d = nc.dram_tensor("w1l2", [H, W1COL], DT, kind="ExternalInput")
    w2l2_d = nc.dram_tensor("w2l2", [H, E], DT, kind="ExternalInput")
    w0l2_d = nc.dram_tensor("w0l2", [H, K], DT, kind="ExternalInput")
    bias_t_d = nc.dram_tensor("bias_t", [128, 8], F32, kind="ExternalInput")
    brow_small_d = nc.dram_tensor("brow_small", [1, SMALL], DT, kind="ExternalInput")
    brow_w1_d = nc.dram_tensor("brow_w1", [1, W1COL], DT, kind="ExternalInput")
    brow_w2_d = nc.dram_tensor("brow_w2", [1, E], DT, kind="ExternalInput")
    brow_w0_d = nc.dram_tensor("brow_w0", [1, K], DT, kind="ExternalInput")
    crx_d = nc.dram_tensor("crx", [128, C * JW], DT, kind="ExternalInput")
    qvb_d = nc.dram_tensor("qvb", [128, C * NA], DT, kind="ExternalInput")
    qvf_d = nc.dram_tensor("qvf", [128, C * NA], F32, kind="ExternalInput")
    dmask_d = nc.dram_tensor("dmask", [128, NV], F32, kind="ExternalInput")
    consts_d = nc.dram_tensor("consts", [128, E + 2], F32, kind="ExternalInput")
    out_d = nc.dram_tensor("out", [128, C], F32, kind="ExternalOutput")

    with tile.TileContext(nc) as tc, ExitStack() as ctx:
        pool = ctx.enter_context(tc.tile_pool(name="sbuf", bufs=1))
        psum = ctx.enter_context(tc.tile_pool(name="psum", bufs=2, space="PSUM"))

        # ---- input loads -------------------------------------------------
        s_t_s = pool.tile([128, 4, R], DT)
        wcat_s = pool.tile([128, 4, 768], DT)
        wsmall_s = pool.tile([128, 4, SMALL], DT)
        for kc in range(4):
            nc.sync.dma_start(s_t_s[:, kc, :], s_t_d[kc * 128:(kc + 1) * 128, :])
            nc.sync.dma_start(wcat_s[:, kc, :], wcat_d[kc * 128:(kc + 1) * 128, :])
            nc.sync.dma_start(wsmall_s[:, kc, :], wsmall_d[kc * 128:(kc + 1) * 128, :])
        w1l2_s = pool.tile([128, 2, W1COL], DT)
        w2l2_s = pool.tile([128, 2, E], DT)
        w0l2_s = pool.tile([128, 2, K], DT)
        for kc in range(2):
            nc.sync.dma_start(w1l2_s[:, kc, :], w1l2_d[kc * 128:(kc + 1) * 128, :])
            nc.sync.dma_start(w2l2_s[:, kc, :], w2l2_d[kc * 128:(kc + 1) * 128, :])
            nc.sync.dma_start(w0l2_s[:, kc, :], w0l2_d[kc * 128:(kc + 1) * 128, :])
        bias_t_s = pool.tile([128, 8], F32)
        nc.sync.dma_start(bias_t_s[:], bias_t_d[:])
        brow_small_s = pool.tile([1, SMALL], DT)
        brow_w1_s = pool.tile([1, W1COL], DT)
        brow_w2_s = pool.tile([1, E], DT)
        brow_w0_s = pool.tile([1, K], DT)
        nc.sync.dma_start(brow_small_s[:], brow_small_d[:])
        nc.sync.dma_start(brow_w1_s[:], brow_w1_d[:])
        nc.sync.dma_start(brow_w2_s[:], brow_w2_d[:])
        nc.sync.dma_start(brow_w0_s[:], brow_w0_d[:])
        crx_s = pool.tile([128, C, JW], DT)
        nc.sync.dma_start(crx_s[:].rearrange("p c j -> p (c j)"), crx_d[:])
        qvb_s = pool.tile([128, C, NA], DT)
        nc.sync.dma_start(qvb_s[:].rearrange("p c j -> p (c j)"), qvb_d[:])
        qvf_s = pool.tile([128, C, NA], F32)
        nc.sync.dma_start(qvf_s[:].rearrange("p c j -> p (c j)"), qvf_d[:])
        dmask_s = pool.tile([128, NV], F32)
        nc.sync.dma_start(dmask_s[:], dmask_d[:])
        consts_s = pool.tile([128, E + 2], F32)
        nc.sync.dma_start(consts_s[:], consts_d[:])
        ones_s = pool.tile([1, 128], DT)
        nc.vector.memset(ones_s[:], 1.0)

        def bc(ap, like):
            a, _ = broadcast_tensor_aps(ap, like)
            return a

        # ---- gather: acc[p,c,j] = qv[p,c, crx[p,c,j]] --------------------
        acc_s = pool.tile([128, C, JW], DT)
        mask_s = pool.tile([128, C, JW], U16)
        for a in range(NA):
            nc.vector.tensor_scalar(mask_s[:], crx_s[:], float(a), None, OP.is_equal)
            data = bc(qvb_s[:, :, a:a + 1], mask_s[:])
            if a == 0:
                nc.vector.tensor_tensor(acc_s[:], mask_s[:], data, OP.mult)
            else:
                nc.vector.copy_predicated(acc_s[:], mask_s[:], data)

        # ---- stage-1 transposed GEMMs: z = relu(Wcat.T @ states + b) ----
        z_s = pool.tile([128, 6, R], DT)      # zA | z1 | z2 (feat-major)
        zad_s = pool.tile([128, 2, 128], DT)  # diag zA, chunk 0 only
        for fc in range(6):
            for rh in range(2):
                p1 = psum.tile([128, 512], F32, tag="s1t")
                for kc in range(4):
                    nc.tensor.matmul(
                        p1[:], wcat_s[:, kc, fc * 128:(fc + 1) * 128],
                        s_t_s[:, kc, rh * 512:(rh + 1) * 512],
                        start=(kc == 0), stop=(kc == 3))
                nc.scalar.activation(z_s[:, fc, rh * 512:(rh + 1) * 512], p1[:],
                                     AF.Relu, bias=bias_t_s[:, fc:fc + 1])
                if rh == 0 and fc < 2:
                    nc.scalar.activation(zad_s[:, fc, :], p1[:, 0:128],
                                         AF.Relu, bias=bias_t_s[:, 6 + fc:7 + fc])

        # ---- stage-1 row-major small heads ------------------------------
        small_s = pool.tile([128, C, SMALL], F32)  # w01|b01|b00|b1|zb2(pre-relu)
        for rc in range(C):
            p2 = psum.tile([128, SMALL], F32, tag="s1r")
            nc.tensor.matmul(p2[:], ones_s[:], brow_small_s[:], start=True, stop=False)
            for kc in range(4):
                nc.tensor.matmul(p2[:], s_t_s[:, kc, rc * 128:(rc + 1) * 128],
                                 wsmall_s[:, kc, :], start=False, stop=(kc == 3))
            nc.vector.tensor_copy(small_s[:, rc, :], p2[:])
        zb2r_s = pool.tile([128, C, E], F32)
        nc.vector.tensor_single_scalar(zb2r_s[:], small_s[:, :, 44:76], 0.0, OP.max)

        # ---- stage-2 row-major GEMMs ------------------------------------
        w1r_s = pool.tile([128, C, W1COL], DT)  # |w1| e-major: col = e*17+v
        for rc in range(C):
            for h in range(2):
                p3 = psum.tile([128, 272], F32, tag="w1")
                nc.tensor.matmul(p3[:], ones_s[:], brow_w1_s[:, h * 272:(h + 1) * 272],
                                 start=True, stop=False)
                for kc in range(2):
                    nc.tensor.matmul(p3[:], z_s[:, 2 + kc, rc * 128:(rc + 1) * 128],
                                     w1l2_s[:, kc, h * 272:(h + 1) * 272],
                                     start=False, stop=(kc == 1))
                nc.scalar.activation(w1r_s[:, rc, h * 272:(h + 1) * 272], p3[:], AF.Abs)
        w2r_s = pool.tile([128, C, E], DT)    # |w2|
        w0c_s = pool.tile([128, C, K], DT)    # |w0_common|
        for rc in range(C):
            p4 = psum.tile([128, E], F32, tag="w20")
            nc.tensor.matmul(p4[:], ones_s[:], brow_w2_s[:], start=True, stop=False)
            for kc in range(2):
                nc.tensor.matmul(p4[:], z_s[:, 4 + kc, rc * 128:(rc + 1) * 128],
                                 w2l2_s[:, kc, :], start=False, stop=(kc == 1))
            nc.scalar.activation(w2r_s[:, rc, :], p4[:], AF.Abs)
            p5 = psum.tile([128, K], F32, tag="w20")
            nc.tensor.matmul(p5[:], ones_s[:], brow_w0_s[:], start=True, stop=False)
            for kc in range(2):
                nc.tensor.matmul(p5[:], z_s[:, 0 + kc, rc * 128:(rc + 1) * 128],
                                 w0l2_s[:, kc, :], start=False, stop=(kc == 1))
            nc.scalar.activation(w0c_s[:, rc, :], p5[:], AF.Abs)
        w0d_s = pool.tile([128, K], DT)       # |w0_diag|, chunk 0
        p6 = psum.tile([128, K], F32, tag="w20")
        nc.tensor.matmul(p6[:], ones_s[:], brow_w0_s[:], start=True, stop=False)
        for kc in range(2):
            nc.tensor.matmul(p6[:], zad_s[:, kc, :], w0l2_s[:, kc, :],
                             start=False, stop=(kc == 1))
        nc.scalar.activation(w0d_s[:], p6[:], AF.Abs)

        # ---- group values ------------------------------------------------
        gath4 = acc_s[:, :, 0:NVK].rearrange("p c (v k) -> p c v k", k=K)
        w04 = w0c_s[:].rearrange("p c (o k) -> p c o k", o=1)
        prodg_s = pool.tile([128, C, NV, K], DT)
        nc.vector.tensor_tensor(prodg_s[:], gath4, bc(w04, gath4), OP.mult)
        group_s = pool.tile([128, C, NV], F32)
        nc.vector.tensor_reduce(group_s[:], prodg_s[:], AX.X, OP.add)
        gb = small_s[:, :, 11:12]
        nc.vector.tensor_tensor(group_s[:], group_s[:], bc(gb, group_s[:]), OP.add)

        # ---- diag correction (chunk 0) ----------------------------------
        dw_s = pool.tile([128, K], F32)
        nc.vector.tensor_tensor(dw_s[:], w0d_s[:], w0c_s[:, 0, :], OP.subtract)
        gselp_s = pool.tile([128, K], F32)
        nc.vector.tensor_tensor(gselp_s[:], acc_s[:, 0, NVK:JW], dw_s[:], OP.mult)
        corr0_s = pool.tile([128, 1], F32)
        nc.vector.tensor_reduce(corr0_s[:], gselp_s[:], AX.X, OP.add)
        corr_s = pool.tile([128, 1], F32)
        nc.vector.tensor_scalar(corr_s[:], corr0_s[:], consts_s[:, E:E + 1], None, OP.add)
        nc.vector.scalar_tensor_tensor(group_s[:, 0, :], dmask_s[:], corr_s[:],
                                       group_s[:, 0, :], OP.mult, OP.add)

        # ---- "other" residual head --------------------------------------
        prodo_s = pool.tile([128, C, NA], F32)
        nc.vector.tensor_tensor(prodo_s[:], qvf_s[:], small_s[:, :, 0:NA], OP.mult)
        other_s = pool.tile([128, C], F32)
        nc.vector.tensor_reduce(other_s[:], prodo_s[:], AX.X, OP.add)
        nc.vector.tensor_tensor(other_s[:], other_s[:], small_s[:, :, NA], OP.add)

        # ---- gq assembly -------------------------------------------------
        gq_s = pool.tile([128, C, NV + 1], DT)
        nc.vector.tensor_copy(gq_s[:, :, 0:NV], group_s[:])
        nc.vector.tensor_copy(gq_s[:, :, NV:NV + 1],
                              other_s[:].rearrange("p (c o) -> p c o", o=1))

        # ---- hidden = elu(sum_v gq_v * |w1|_{v,e} + b1) ------------------
        w1v = w1r_s[:].rearrange("p c (e v) -> p c e v", v=NV + 1)
        gqv = gq_s[:].rearrange("p c (o v) -> p c o v", o=1)
        prodh_s = pool.tile([128, C, E, NV + 1], DT)
        nc.vector.tensor_tensor(prodh_s[:], w1v, bc(gqv, w1v), OP.mult)
        mix_s = pool.tile([128, C, E], F32)
        nc.vector.tensor_reduce(mix_s[:], prodh_s[:], AX.X, OP.add)
        hidp_s = pool.tile([128, C, E], F32)
        nc.vector.tensor_tensor(hidp_s[:], mix_s[:], small_s[:, :, 12:44], OP.add)
        m_s = pool.tile([128, C, E], F32)
        nc.vector.tensor_single_scalar(m_s[:], hidp_s[:], 0.0, OP.min)
        e_s = pool.tile([128, C, E], F32)
        nc.scalar.activation(e_s[:], m_s[:], AF.Exp)
        hid_s = pool.tile([128, C, E], F32)   # = elu(hidp) + 1
        nc.vector.scalar_tensor_tensor(hid_s[:], hidp_s[:], 0.0, e_s[:], OP.max, OP.add)

        # ---- y = sum_e (hid-1)*|w2| + b2 --------------------------------
        prodf_s = pool.tile([128, C, E], F32)
        nc.vector.tensor_tensor(prodf_s[:], hid_s[:], w2r_s[:], OP.mult)
        ysum_s = pool.tile([128, C], F32)
        nc.vector.tensor_reduce(ysum_s[:], prodf_s[:], AX.X, OP.add)
        w2sum_s = pool.tile([128, C], F32)
        nc.vector.tensor_reduce(w2sum_s[:], w2r_s[:], AX.X, OP.add)
        b2p_s = pool.tile([128, C, E], F32)
        cb2 = consts_s[:, 0:E].rearrange("p (o e) -> p o e", o=1)
        nc.vector.tensor_tensor(b2p_s[:], zb2r_s[:], bc(cb2, zb2r_s[:]), OP.mult)
        b2v_s = pool.tile([128, C], F32)
        nc.vector.tensor_reduce(b2v_s[:], b2p_s[:], AX.X, OP.add)
        y_s = pool.tile([128, C], F32)
        nc.vector.tensor_tensor(y_s[:], ysum_s[:], w2sum_s[:], OP.subtract)
        nc.vector.tensor_tensor(y_s[:], y_s[:], b2v_s[:], OP.add)
        nc.vector.tensor_scalar(y_s[:], y_s[:], consts_s[:, E + 1:E + 2], None, OP.add)
        nc.sync.dma_start(out_d[:], y_s[:])

    nc.compile()
    return nc


def _prep_inputs(inputs):
    g = lambda k: np.asarray(inputs[k], dtype=np.float32)
    states = g('states')
    qvals = g('qvals')
    cr = np.asarray(inputs['causal_relations'])

    w00_l1_W, w00_l1_b = g('w00_l1_W'), g('w00_l1_b')
    b00_W, b00_b = g('b00_W'), g('b00_b')
    h_delta = w00_l1_W[SD:].sum(0)
    g_delta = float(b00_W[SD:].sum(0)[0])

    wcat = np.concatenate([w00_l1_W[:SD], g('w1_l1_W'), g('w2_l1_W')], axis=1)
    b_cat = np.concatenate([w00_l1_b, g('w1_l1_b'), g('w2_l1_b')])
    bias_t = np.zeros((128, 8), np.float32)
    for fc in range(6):
        bias_t[:, fc] = b_cat[fc * 128:(fc + 1) * 128]
    for fc in range(2):
        bias_t[:, 6 + fc] = (w00_l1_b + h_delta)[fc * 128:(fc + 1) * 128]

    wsmall = np.concatenate([g('w01_W'), g('b01_W'), b00_W[:SD],
                             g('b1_W'), g('b2_l1_W')], axis=1)
    brow_small = np.concatenate([g('w01_b'), g('b01_b'), b00_b,
                                 g('b1_b'), g('b2_l1_b')])[None, :]

    perm = np.array([v * E + e for e in range(E) for v in range(NV + 1)])
    w1l2 = g('w1_l2_W')[:, perm]
    brow_w1 = g('w1_l2_b')[perm][None, :]
    w2l2, brow_w2 = g('w2_l2_W'), g('w2_l2_b')[None, :]
    w0l2, brow_w0 = g('w00_l2_W'), g('w00_l2_b')[None, :]

    shared = dict(
        wcat=wcat.astype(BF), wsmall=wsmall.astype(BF),
        w1l2=w1l2.astype(BF), w2l2=w2l2.astype(BF), w0l2=w0l2.astype(BF),
        bias_t=bias_t,
        brow_small=brow_small.astype(BF), brow_w1=brow_w1.astype(BF),
        brow_w2=brow_w2.astype(BF), brow_w0=brow_w0.astype(BF),
    )

    in_maps = []
    for m in range(NCORES):
        bs = m + 8 * np.arange(16)
        S2 = states[bs].reshape(R, SD)
        s_t = np.ascontiguousarray(S2.T).astype(BF)

        qv = qvals[bs].reshape(R, NA)           # [r, a], r = bi*64+t
        cr_vk = np.swapaxes(cr[bs].reshape(R, K, NV), 1, 2)  # [r, v, k]
        crx = np.zeros((R, JW), np.float32)
        crx[:, 0:NVK] = cr_vk.reshape(R, NVK)
        vd = np.where(np.arange(128) < 64, m, m + 8)
        crx[0:128, NVK:JW] = cr_vk[np.arange(128), vd, :]
        # row r = c*128+p lives at tile[p, c]
        to_pc = lambda x: np.ascontiguousarray(
            x.reshape(C, 128, -1).transpose(1, 0, 2).reshape(128, -1))
        dmask = np.zeros((128, NV), np.float32)
        dmask[np.arange(128), vd] = 1.0
        consts = np.zeros((128, E + 2), np.float32)
        consts[:, 0:E] = g('b2_l2_W')[:, 0][None, :]
        consts[:, E] = g_delta
        consts[:, E + 1] = float(g('b2_l2_b')[0])
        in_maps.append(dict(shared,
                            s_t=s_t, crx=to_pc(crx).astype(BF),
                            qvb=to_pc(qv).astype(BF), qvf=to_pc(qv),
                            dmask=dmask, consts=consts))
    return in_maps


def kernel(**inputs):
    if 'nc' not in _cache:
        _cache['nc'] = _build_nc()
    nc = _cache['nc']
    in_maps = _prep_inputs(inputs)
    res = run_bass_kernel_spmd(nc, in_maps, list(range(NCORES)),
                               **_cache.get('run_kwargs', {}))
    _cache['last_result'] = res
    y = np.zeros((B, T, 1), np.float32)
    for m in range(NCORES):
        bs = m + 8 * np.arange(16)
        o = res.results[m]['out']               # [128, C]
        rows = np.ascontiguousarray(o.T).reshape(R)   # r = c*128+p
        y[bs] = rows.reshape(16, T, 1)
    return y


# revision 5
# speedup vs baseline: 1.1635x; 1.1635x over previous
"""Trainium2 Bass kernel for nn_CausalMixer (QMIX-style causal mixer).

Data-parallel across 8 NeuronCores: batch dim sharded round-robin
(core m gets batches m, m+8, m+16, ...), hypernet weights replicated.

Per-core layout (R = 1024 rows = 16 batches x 64 timesteps):
  - stage-1 "transposed" GEMMs: out[feat, rows] = Wcat.T-chunks @ states.T,
    evicted with fused per-partition bias+ReLU on ScalarE.
  - stage-2 row-major GEMMs: the relu'd z tiles [feat, rows] serve directly
    as lhsT, producing per-row hypernet weights [rows, feat]; bias is
    preloaded into PSUM with a K=1 ones-matmul.
  - gather (qvals[cr]) via 10x {is_equal mask + copy_predicated} on DVE.
  - the onehot quirk (batch row b==v gets +delta) is handled as a rank-1
    correction on chunk 0 only (host orders the diag batches first).
  - all bf16 inputs ride one mega-packed DRAM tensor (few big DMAs,
    need-ordered columns); dummy matmuls at t=0 lift the PE HAM throttle.
"""

import sys

for _p in ("/root/.axon_site", "/root/.axon_site/_ro/trn_rl_repo",
           "/root/.axon_site/_ro/pypackages", "/opt/trn_rl_repo"):
    if _p not in sys.path:
        sys.path.append(_p)

import numpy as np
import ml_dtypes
from contextlib import ExitStack

import concourse.bass as bass
import concourse.bacc as bacc_mod
import concourse.tile as tile
import concourse.mybir as mybir
from concourse.bass import broadcast_tensor_aps
from concourse.bass_utils import run_bass_kernel_spmd

BF = ml_dtypes.bfloat16
DT = mybir.dt.bfloat16
F32 = mybir.dt.float32
U16 = mybir.dt.uint16
OP = mybir.AluOpType
AF = mybir.ActivationFunctionType
AX = mybir.AxisListType

NCORES = 8
B, T, NA, NV, K, SD, H, E = 128, 64, 10, 16, 4, 512, 256, 32
R = 16 * T            # rows per core = 1024
C = R // 128          # row chunks per core = 8
NVK = NV * K          # 64
JW = NVK + K          # 68 gather cols (64 + 4 diag)
SMALL = NA + 1 + 1 + E + E   # 76: w01 | b01 | b00 | b1 | b2_l1
W1COL = (NV + 1) * E  # 544

# ---- mega-packed bf16 input column map (need-ordered) -------------------
# 4x [wcat_kc (768) | s_t_kc (1024)] pairs, then tail:
#   wsmall 4x76 | w1l2 2x544 | w2l2 2x32 | w0l2 2x4 | crx 8x68 | qvb 8x10
#   | brow_small 76 | brow_w1 544 | brow_w2 32 | brow_w0 4  (partition 0)
PAIR = 768 + R
OFF_TAIL = 4 * PAIR
OFF_WSMALL = OFF_TAIL
OFF_W1L2 = OFF_WSMALL + 4 * SMALL
OFF_W2L2 = OFF_W1L2 + 2 * W1COL
OFF_W0L2 = OFF_W2L2 + 2 * E
OFF_CRX = OFF_W0L2 + 2 * K
OFF_QVB = OFF_CRX + C * JW
OFF_BROW = OFF_QVB + C * NA
NBF = OFF_BROW + SMALL + W1COL + E + K
# f32 mega: bias_t (8) | dmask (16) | consts (34)
GOFF_BIAS = 0
GOFF_DMASK = 8
GOFF_CONSTS = 24
NF32 = GOFF_CONSTS + E + 2

_cache = {}


def _build_nc():
    nc = bacc_mod.Bacc("TRN2", target_bir_lowering=False, debug=False)

    mb_d = nc.dram_tensor("mb", [128, NBF], DT, kind="ExternalInput")
    mf_d = nc.dram_tensor("mf", [128, NF32], F32, kind="ExternalInput")
    out_d = nc.dram_tensor("out", [128, C], F32, kind="ExternalOutput")

    with tile.TileContext(nc) as tc, ExitStack() as ctx:
        pool = ctx.enter_context(tc.tile_pool(name="sbuf", bufs=1))
        hpool = ctx.enter_context(tc.tile_pool(name="hbuf", bufs=3))
        psum = ctx.enter_context(tc.tile_pool(name="psum", bufs=2, space="PSUM"))

        mb_s = pool.tile([128, NBF], DT)
        mf_s = pool.tile([128, NF32], F32)
        # need-ordered loads: one DMA per (wcat,s_t) kc pair, one tail, one f32
        for kc in range(4):
            nc.sync.dma_start(mb_s[:, kc * PAIR:(kc + 1) * PAIR],
                              mb_d[:, kc * PAIR:(kc + 1) * PAIR])
        nc.sync.dma_start(mb_s[:, OFF_TAIL:NBF], mb_d[:, OFF_TAIL:NBF])
        nc.sync.dma_start(mf_s[:], mf_d[:])

        def wcat(kc, c0, c1):
            return mb_s[:, kc * PAIR + c0:kc * PAIR + c1]

        def s_t(kc, c0, c1):
            return mb_s[:, kc * PAIR + 768 + c0:kc * PAIR + 768 + c1]

        def wsmall(kc):
            return mb_s[:, OFF_WSMALL + kc * SMALL:OFF_WSMALL + (kc + 1) * SMALL]

        def w1l2(kc, c0, c1):
            return mb_s[:, OFF_W1L2 + kc * W1COL + c0:OFF_W1L2 + kc * W1COL + c1]

        def w2l2(kc):
            return mb_s[:, OFF_W2L2 + kc * E:OFF_W2L2 + (kc + 1) * E]

        def w0l2(kc):
            return mb_s[:, OFF_W0L2 + kc * K:OFF_W0L2 + (kc + 1) * K]

        crx_s = mb_s[:, OFF_CRX:OFF_CRX + C * JW].rearrange(
            "p (c j) -> p c j", j=JW)
        qvb_s = mb_s[:, OFF_QVB:OFF_QVB + C * NA].rearrange(
            "p (c j) -> p c j", j=NA)
        brow_small_s = mb_s[0:1, OFF_BROW:OFF_BROW + SMALL]
        brow_w1_s = mb_s[0:1, OFF_BROW + SMALL:OFF_BROW + SMALL + W1COL]
        brow_w2_s = mb_s[0:1, OFF_BROW + SMALL + W1COL:
                         OFF_BROW + SMALL + W1COL + E]
        brow_w0_s = mb_s[0:1, OFF_BROW + SMALL + W1COL + E:
                         OFF_BROW + SMALL + W1COL + E + K]
        bias_t_s = mf_s[:, GOFF_BIAS:GOFF_BIAS + 8]
        dmask_s = mf_s[:, GOFF_DMASK:GOFF_DMASK + NV]
        consts_s = mf_s[:, GOFF_CONSTS:GOFF_CONSTS + E + 2]

        ones_s = pool.tile([1, 128], DT)
        nc.vector.memset(ones_s[:], 1.0)

        # ---- PE warmup: dummy matmuls lift the HAM clock gate -----------
        warm_s = pool.tile([128, 512], DT)
        nc.vector.memset(warm_s[:, 0:128], 0.0)
        for i in range(16):
            pw = psum.tile([128, 512], F32, tag="s1t")
            nc.tensor.matmul(pw[:], warm_s[:, 0:128], warm_s[:],
                             start=True, stop=True)

        def bc(ap, like):
            a, _ = broadcast_tensor_aps(ap, like)
            return a

        # ---- gather: acc[p,c,j] = qv[p,c, crx[p,c,j]] (DVE, overlaps PE) --
        acc_s = pool.tile([128, C, JW], DT)
        mask_s = pool.tile([128, C, JW], U16)
        for a in range(NA):
            nc.vector.tensor_scalar(mask_s[:], crx_s, float(a), None, OP.is_equal)
            data = bc(qvb_s[:, :, a:a + 1], mask_s[:])
            if a == 0:
                nc.vector.tensor_tensor(acc_s[:], mask_s[:], data, OP.mult)
            else:
                nc.vector.copy_predicated(acc_s[:], mask_s[:], data)

        # ---- stage-1 transposed GEMMs: z = relu(Wcat.T @ states + b) ----
        z_s = pool.tile([128, 6, R], DT)      # zA | z1 | z2 (feat-major)
        zad_s = pool.tile([128, 2, 128], DT)  # diag zA, chunk 0 only
        for fc in range(6):
            for rh in range(2):
                p1 = psum.tile([128, 512], F32, tag="s1t")
                for kc in range(4):
                    nc.tensor.matmul(
                        p1[:], wcat(kc, fc * 128, (fc + 1) * 128),
                        s_t(kc, rh * 512, (rh + 1) * 512),
                        start=(kc == 0), stop=(kc == 3))
                nc.scalar.activation(z_s[:, fc, rh * 512:(rh + 1) * 512], p1[:],
                                     AF.Relu, bias=bias_t_s[:, fc:fc + 1])
                if rh == 0 and fc < 2:
                    nc.scalar.activation(zad_s[:, fc, :], p1[:, 0:128],
                                         AF.Relu, bias=bias_t_s[:, 6 + fc:7 + fc])

        # ---- stage-1 row-major small heads ------------------------------
        small_s = pool.tile([128, C, SMALL], F32)  # w01|b01|b00|b1|zb2(pre-relu)
        for rc in range(C):
            p2 = psum.tile([128, SMALL], F32, tag="s1r")
            nc.tensor.matmul(p2[:], ones_s[:], brow_small_s, start=True, stop=False)
            for kc in range(4):
                nc.tensor.matmul(p2[:], s_t(kc, rc * 128, (rc + 1) * 128),
                                 wsmall(kc), start=False, stop=(kc == 3))
            nc.scalar.copy(small_s[:, rc, :], p2[:])
        zb2r_s = pool.tile([128, C, E], F32)
        nc.vector.tensor_single_scalar(zb2r_s[:], small_s[:, :, 44:76], 0.0, OP.max)

        # ---- stage-2: w2 / w0c / w0d first (small, unblock group chain) --
        w2r_s = pool.tile([128, C, E], DT)    # |w2|
        w0c_s = pool.tile([128, C, K], DT)    # |w0_common|
        for rc in range(C):
            p4 = psum.tile([128, E], F32, tag="w20")
            nc.tensor.matmul(p4[:], ones_s[:], brow_w2_s, start=True, stop=False)
            for kc in range(2):
                nc.tensor.matmul(p4[:], z_s[:, 4 + kc, rc * 128:(rc + 1) * 128],
                                 w2l2(kc), start=False, stop=(kc == 1))
            nc.scalar.activation(w2r_s[:, rc, :], p4[:], AF.Abs)
            p5 = psum.tile([128, K], F32, tag="w20")
            nc.tensor.matmul(p5[:], ones_s[:], brow_w0_s, start=True, stop=False)
            for kc in range(2):
                nc.tensor.matmul(p5[:], z_s[:, 0 + kc, rc * 128:(rc + 1) * 128],
                                 w0l2(kc), start=False, stop=(kc == 1))
            nc.scalar.activation(w0c_s[:, rc, :], p5[:], AF.Abs)
        w0d_s = pool.tile([128, K], DT)       # |w0_diag|, chunk 0
        p6 = psum.tile([128, K], F32, tag="w20")
        nc.tensor.matmul(p6[:], ones_s[:], brow_w0_s, start=True, stop=False)
        for kc in range(2):
            nc.tensor.matmul(p6[:], zad_s[:, kc, :], w0l2(kc),
                             start=False, stop=(kc == 1))
        nc.scalar.activation(w0d_s[:], p6[:], AF.Abs)

        # ---- group values ------------------------------------------------
        gath4 = acc_s[:, :, 0:NVK].rearrange("p c (v k) -> p c v k", k=K)
        w04 = w0c_s[:].rearrange("p c (o k) -> p c o k", o=1)
        prodg_s = pool.tile([128, C, NV, K], DT)
        nc.vector.tensor_tensor(prodg_s[:], gath4, bc(w04, gath4), OP.mult)
        group_s = pool.tile([128, C, NV], F32)
        nc.vector.tensor_reduce(group_s[:], prodg_s[:], AX.X, OP.add)
        gb = small_s[:, :, 11:12]
        nc.vector.tensor_tensor(group_s[:], group_s[:], bc(gb, group_s[:]), OP.add)

        # ---- diag correction (chunk 0) ----------------------------------
        dw_s = pool.tile([128, K], F32)
        nc.vector.tensor_tensor(dw_s[:], w0d_s[:], w0c_s[:, 0, :], OP.subtract)
        gselp_s = pool.tile([128, K], F32)
        nc.vector.tensor_tensor(gselp_s[:], acc_s[:, 0, NVK:JW], dw_s[:], OP.mult)
        corr0_s = pool.tile([128, 1], F32)
        nc.vector.tensor_reduce(corr0_s[:], gselp_s[:], AX.X, OP.add)
        corr_s = pool.tile([128, 1], F32)
        nc.vector.tensor_scalar(corr_s[:], corr0_s[:],
                                consts_s[:, E:E + 1], None, OP.add)
        nc.vector.scalar_tensor_tensor(group_s[:, 0, :], dmask_s, corr_s[:],
                                       group_s[:, 0, :], OP.mult, OP.add)

        # ---- "other" residual head --------------------------------------
        prodo_s = pool.tile([128, C, NA], F32)
        nc.vector.tensor_tensor(prodo_s[:], qvb_s, small_s[:, :, 0:NA], OP.mult)
        other_s = pool.tile([128, C], F32)
        nc.vector.tensor_reduce(other_s[:], prodo_s[:], AX.X, OP.add)
        nc.vector.tensor_tensor(other_s[:], other_s[:], small_s[:, :, NA], OP.add)

        # ---- gq assembly -------------------------------------------------
        gq_s = pool.tile([128, C, NV + 1], DT)
        nc.vector.tensor_copy(gq_s[:, :, 0:NV], group_s[:])
        nc.vector.tensor_copy(gq_s[:, :, NV:NV + 1],
                              other_s[:].rearrange("p (c o) -> p c o", o=1))

        # ---- stage-2 w1 GEMMs, fused per-rc with the hidden mix ---------
        # w1r cols are e-major (col = e*17+v); mix[p,rc,e] = sum_v gq*|w1|
        w1r_s = pool.tile([128, C, W1COL], DT)
        mix_s = pool.tile([128, C, E], F32)
        for rc in range(C):
            for h in range(2):
                p3 = psum.tile([128, 272], F32, tag="w1")
                nc.tensor.matmul(p3[:], ones_s[:],
                                 brow_w1_s[:, h * 272:(h + 1) * 272],
                                 start=True, stop=False)
                for kc in range(2):
                    nc.tensor.matmul(p3[:], z_s[:, 2 + kc, rc * 128:(rc + 1) * 128],
                                     w1l2(kc, h * 272, (h + 1) * 272),
                                     start=False, stop=(kc == 1))
                nc.scalar.activation(w1r_s[:, rc, h * 272:(h + 1) * 272],
                                     p3[:], AF.Abs)
            w1v = w1r_s[:, rc, :].rearrange("p (e v) -> p e v", v=NV + 1)
            gqv = gq_s[:, rc, :].rearrange("p (o v) -> p o v", o=1)
            prodh = hpool.tile([128, E, NV + 1], DT, tag="prodh")
            eng = nc.vector if rc % 2 == 0 else nc.gpsimd
            eng.tensor_tensor(prodh[:], w1v, bc(gqv, w1v), OP.mult)
            nc.vector.tensor_reduce(mix_s[:, rc, :], prodh[:], AX.X, OP.add)

        # ---- hidden = elu(mix + b1), y = sum_e (hid-1)*|w2| + b2 --------
        hidp_s = pool.tile([128, C, E], F32)
        nc.vector.tensor_tensor(hidp_s[:], mix_s[:], small_s[:, :, 12:44], OP.add)
        m_s = pool.tile([128, C, E], F32)
        nc.vector.tensor_single_scalar(m_s[:], hidp_s[:], 0.0, OP.min)
        e_s = pool.tile([128, C, E], F32)
        nc.scalar.activation(e_s[:], m_s[:], AF.Exp)
        hid_s = pool.tile([128, C, E], F32)   # = elu(hidp) + 1
        nc.vector.scalar_tensor_tensor(hid_s[:], hidp_s[:], 0.0, e_s[:],
                                       OP.max, OP.add)
        prodf_s = pool.tile([128, C, E], F32)
        nc.vector.tensor_tensor(prodf_s[:], hid_s[:], w2r_s[:], OP.mult)
        ysum_s = pool.tile([128, C], F32)
        nc.vector.tensor_reduce(ysum_s[:], prodf_s[:], AX.X, OP.add)
        w2sum_s = pool.tile([128, C], F32)
        nc.vector.tensor_reduce(w2sum_s[:], w2r_s[:], AX.X, OP.add)
        b2p_s = pool.tile([128, C, E], F32)
        cb2 = consts_s[:, 0:E].rearrange("p (o e) -> p o e", o=1)
        nc.vector.tensor_tensor(b2p_s[:], zb2r_s[:], bc(cb2, zb2r_s[:]), OP.mult)
        b2v_s = pool.tile([128, C], F32)
        nc.vector.tensor_reduce(b2v_s[:], b2p_s[:], AX.X, OP.add)
        y_s = pool.tile([128, C], F32)
        nc.vector.tensor_tensor(y_s[:], ysum_s[:], w2sum_s[:], OP.subtract)
        nc.vector.tensor_tensor(y_s[:], y_s[:], b2v_s[:], OP.add)
        nc.vector.tensor_scalar(y_s[:], y_s[:], consts_s[:, E + 1:E + 2],
                                None, OP.add)
        nc.sync.dma_start(out_d[:], y_s[:])

    nc.compile()
    return nc


def _prep_inputs(inputs):
    g = lambda k: np.asarray(inputs[k], dtype=np.float32)
    states = g('states')
    qvals = g('qvals')
    cr = np.asarray(inputs['causal_relations'])

    w00_l1_W, w00_l1_b = g('w00_l1_W'), g('w00_l1_b')
    b00_W, b00_b = g('b00_W'), g('b00_b')
    h_delta = w00_l1_W[SD:].sum(0)
    g_delta = float(b00_W[SD:].sum(0)[0])

    wcat = np.concatenate([w00_l1_W[:SD], g('w1_l1_W'), g('w2_l1_W')], axis=1)
    b_cat = np.concatenate([w00_l1_b, g('w1_l1_b'), g('w2_l1_b')])
    bias_t = np.zeros((128, 8), np.float32)
    for fc in range(6):
        bias_t[:, fc] = b_cat[fc * 128:(fc + 1) * 128]
    for fc in range(2):
        bias_t[:, 6 + fc] = (w00_l1_b + h_delta)[fc * 128:(fc + 1) * 128]

    wsmall = np.concatenate([g('w01_W'), g('b01_W'), b00_W[:SD],
                             g('b1_W'), g('b2_l1_W')], axis=1)
    brow_small = np.concatenate([g('w01_b'), g('b01_b'), b00_b,
                                 g('b1_b'), g('b2_l1_b')])
    perm = np.array([v * E + e for e in range(E) for v in range(NV + 1)])
    w1l2 = g('w1_l2_W')[:, perm]
    brow_w1 = g('w1_l2_b')[perm]
    w2l2, brow_w2 = g('w2_l2_W'), g('w2_l2_b')
    w0l2, brow_w0 = g('w00_l2_W'), g('w00_l2_b')

    # shared bf16 mega columns (everything except s_t / crx / qvb)
    mb_shared = np.zeros((128, NBF), BF)
    for kc in range(4):
        mb_shared[:, kc * PAIR:kc * PAIR + 768] = wcat[kc * 128:(kc + 1) * 128]
        mb_shared[:, OFF_WSMALL + kc * SMALL:OFF_WSMALL + (kc + 1) * SMALL] = \
            wsmall[kc * 128:(kc + 1) * 128]
    for kc in range(2):
        mb_shared[:, OFF_W1L2 + kc * W1COL:OFF_W1L2 + (kc + 1) * W1COL] = \
            w1l2[kc * 128:(kc + 1) * 128]
        mb_shared[:, OFF_W2L2 + kc * E:OFF_W2L2 + (kc + 1) * E] = \
            w2l2[kc * 128:(kc + 1) * 128]
        mb_shared[:, OFF_W0L2 + kc * K:OFF_W0L2 + (kc + 1) * K] = \
            w0l2[kc * 128:(kc + 1) * 128]
    o = OFF_BROW
    mb_shared[0, o:o + SMALL] = brow_small
    mb_shared[0, o + SMALL:o + SMALL + W1COL] = brow_w1
    mb_shared[0, o + SMALL + W1COL:o + SMALL + W1COL + E] = brow_w2
    mb_shared[0, o + SMALL + W1COL + E:o + SMALL + W1COL + E + K] = brow_w0

    mf_shared = np.zeros((128, NF32), np.float32)
    mf_shared[:, GOFF_BIAS:GOFF_BIAS + 8] = bias_t
    mf_shared[:, GOFF_CONSTS:GOFF_CONSTS + E] = g('b2_l2_W')[:, 0][None, :]
    mf_shared[:, GOFF_CONSTS + E] = g_delta
    mf_shared[:, GOFF_CONSTS + E + 1] = float(g('b2_l2_b')[0])

    to_pc = lambda x: np.ascontiguousarray(
        x.reshape(C, 128, -1).transpose(1, 0, 2).reshape(128, -1))

    in_maps = []
    for m in range(NCORES):
        bs = m + 8 * np.arange(16)
        mb = mb_shared.copy()
        S2 = states[bs].reshape(R, SD)
        s_tT = np.ascontiguousarray(S2.T).astype(BF)    # [512, R]
        for kc in range(4):
            mb[:, kc * PAIR + 768:(kc + 1) * PAIR] = \
                s_tT[kc * 128:(kc + 1) * 128]

        qv = qvals[bs].reshape(R, NA)
        cr_vk = np.swapaxes(cr[bs].reshape(R, K, NV), 1, 2)  # [r, v, k]
        crx = np.zeros((R, JW), np.float32)
        crx[:, 0:NVK] = cr_vk.reshape(R, NVK)
        vd = np.where(np.arange(128) < 64, m, m + 8)
        crx[0:128, NVK:JW] = cr_vk[np.arange(128), vd, :]
        mb[:, OFF_CRX:OFF_CRX + C * JW] = to_pc(crx)
        mb[:, OFF_QVB:OFF_QVB + C * NA] = to_pc(qv)

        mf = mf_shared.copy()
        dmask = np.zeros((128, NV), np.float32)
        dmask[np.arange(128), vd] = 1.0
        mf[:, GOFF_DMASK:GOFF_DMASK + NV] = dmask
        in_maps.append(dict(mb=mb, mf=mf))
    return in_maps


def kernel(**inputs):
    if 'nc' not in _cache:
        _cache['nc'] = _build_nc()
    nc = _cache['nc']
    in_maps = _prep_inputs(inputs)
    res = run_bass_kernel_spmd(nc, in_maps, list(range(NCORES)),
                               **_cache.get('run_kwargs', {}))
    _cache['last_result'] = res
    y = np.zeros((B, T, 1), np.float32)
    for m in range(NCORES):
        bs = m + 8 * np.arange(16)
        o = res.results[m]['out']               # [128, C]
        rows = np.ascontiguousarray(o.T).reshape(R)   # r = c*128+p
        y[bs] = rows.reshape(16, T, 1)
    return y


# revision 10
# speedup vs baseline: 1.1835x; 1.0173x over previous
"""Trainium2 Bass kernel for nn_CausalMixer (QMIX-style causal mixer).

Data-parallel across 8 NeuronCores: batch dim sharded round-robin
(core m gets batches m, m+8, m+16, ...), hypernet weights replicated.

Per-core layout (R = 1024 rows = 16 batches x 64 timesteps):
  - stage-1 "transposed" GEMMs: out[feat, rows] = Wcat.T-chunks @ states.T,
    evicted with fused per-partition bias+ReLU on ScalarE.
  - stage-2 row-major GEMMs: the relu'd z tiles [feat, rows] serve directly
    as lhsT, producing per-row hypernet weights [rows, feat]; bias is
    preloaded into PSUM with a K=1 ones-matmul.
  - gather (qvals[cr]) via 10x {is_equal mask + copy_predicated} on DVE.
  - the onehot quirk (batch row b==v gets +delta) is handled as a rank-1
    correction on chunk 0 only (host orders the diag batches first).
  - all bf16 inputs ride one mega-packed DRAM tensor (few big DMAs,
    need-ordered columns); dummy matmuls at t=0 lift the PE HAM throttle.
"""

import sys

for _p in ("/root/.axon_site", "/root/.axon_site/_ro/trn_rl_repo",
           "/root/.axon_site/_ro/pypackages", "/opt/trn_rl_repo"):
    if _p not in sys.path:
        sys.path.append(_p)

import numpy as np
import ml_dtypes
from contextlib import ExitStack

import concourse.bass as bass
import concourse.bacc as bacc_mod
import concourse.tile as tile
import concourse.mybir as mybir
from concourse.bass import broadcast_tensor_aps
from concourse.bass_utils import run_bass_kernel_spmd

BF = ml_dtypes.bfloat16
DT = mybir.dt.bfloat16
F32 = mybir.dt.float32
U16 = mybir.dt.uint16
OP = mybir.AluOpType
AF = mybir.ActivationFunctionType
AX = mybir.AxisListType

NCORES = 8
B, T, NA, NV, K, SD, H, E = 128, 64, 10, 16, 4, 512, 256, 32
R = 16 * T            # rows per core = 1024
C = R // 128          # row chunks per core = 8
NVK = NV * K          # 64
JW = NVK + K          # 68 gather cols (64 + 4 diag)
SMALL = NA + 1 + 1 + E + E   # 76: w01 | b01 | b00 | b1 | b2_l1
W1COL = (NV + 1) * E  # 544

# ---- mega-packed bf16 input column map (need-ordered) -------------------
# crx 8x68 | qvb 8x10, then 4x [wcat_kc (768) | s_t_kc (1024)] pairs,
# then qvx 8x10x68, then tail:
#   wsmall 4x76 | w1l2 2x544 | w2l2 2x32 | w0l2 2x4
#   | brow_small4 4x76 | brow_w1 544 | brow_w20 36  (partition 0)
PAIR = 768 + R
OFF_CRX = 0
OFF_QVB = OFF_CRX + C * JW
OFF_PAIR = OFF_QVB + C * NA
OFF_QVX = OFF_PAIR + 4 * PAIR
OFF_TAIL = OFF_QVX + C * NA * JW
OFF_WSMALL = OFF_TAIL
OFF_W1L2 = OFF_WSMALL + 4 * SMALL
OFF_W2L2 = OFF_W1L2 + 2 * W1COL
OFF_W0L2 = OFF_W2L2 + 2 * E
OFF_BROW = OFF_W0L2 + 2 * K
NBF = OFF_BROW + 4 * SMALL + W1COL + E + K
# f32 mega: bias_t (8) | dmask (16) | consts (34)
GOFF_BIAS = 0
GOFF_DMASK = 8
GOFF_CONSTS = 24
NF32 = GOFF_CONSTS + E + 2

_cache = {}


def _build_nc():
    nc = bacc_mod.Bacc("TRN2", target_bir_lowering=False, debug=False)

    mb_d = nc.dram_tensor("mb", [128, NBF], DT, kind="ExternalInput")
    mf_d = nc.dram_tensor("mf", [128, NF32], F32, kind="ExternalInput")
    out_d = nc.dram_tensor("out", [128, C], F32, kind="ExternalOutput")

    with tile.TileContext(nc) as tc, ExitStack() as ctx:
        pool = ctx.enter_context(tc.tile_pool(name="sbuf", bufs=1))
        hpool = ctx.enter_context(tc.tile_pool(name="hbuf", bufs=3))
        psum = ctx.enter_context(tc.tile_pool(name="psum", bufs=2, space="PSUM"))

        mb_s = pool.tile([128, NBF], DT)
        mf_s = pool.tile([128, NF32], F32)
        # need-ordered loads: crx/qvb block, (wcat,s_t) kc pairs, qvx, tail, f32
        nc.sync.dma_start(mb_s[:, OFF_CRX:OFF_PAIR], mb_d[:, OFF_CRX:OFF_PAIR])
        for kc in range(4):
            nc.sync.dma_start(
                mb_s[:, OFF_PAIR + kc * PAIR:OFF_PAIR + (kc + 1) * PAIR],
                mb_d[:, OFF_PAIR + kc * PAIR:OFF_PAIR + (kc + 1) * PAIR])
        nc.sync.dma_start(mb_s[:, OFF_QVX:OFF_TAIL], mb_d[:, OFF_QVX:OFF_TAIL])
        nc.sync.dma_start(mb_s[:, OFF_TAIL:NBF], mb_d[:, OFF_TAIL:NBF])
        nc.sync.dma_start(mf_s[:], mf_d[:])

        def wcat(kc, c0, c1):
            return mb_s[:, OFF_PAIR + kc * PAIR + c0:OFF_PAIR + kc * PAIR + c1]

        def s_t(kc, c0, c1):
            return mb_s[:, OFF_PAIR + kc * PAIR + 768 + c0:
                        OFF_PAIR + kc * PAIR + 768 + c1]

        def wsmall(kc):
            return mb_s[:, OFF_WSMALL + kc * SMALL:OFF_WSMALL + (kc + 1) * SMALL]

        def w1l2(kc, c0, c1):
            return mb_s[:, OFF_W1L2 + kc * W1COL + c0:OFF_W1L2 + kc * W1COL + c1]

        def w2l2(kc):
            return mb_s[:, OFF_W2L2 + kc * E:OFF_W2L2 + (kc + 1) * E]

        def w0l2(kc):
            return mb_s[:, OFF_W0L2 + kc * K:OFF_W0L2 + (kc + 1) * K]

        crx_s = mb_s[:, OFF_CRX:OFF_CRX + C * JW].rearrange(
            "p (c j) -> p c j", j=JW)
        qvb_s = mb_s[:, OFF_QVB:OFF_QVB + C * NA].rearrange(
            "p (c j) -> p c j", j=NA)
        qvx_s = mb_s[:, OFF_QVX:OFF_TAIL].rearrange(
            "p (c a j) -> p c a j", a=NA, j=JW)
        brow_small4_s = mb_s[0:1, OFF_BROW:OFF_BROW + 4 * SMALL]
        brow_w1_s = mb_s[0:1, OFF_BROW + 4 * SMALL:OFF_BROW + 4 * SMALL + W1COL]
        brow_w20_s = mb_s[0:1, OFF_BROW + 4 * SMALL + W1COL:
                          OFF_BROW + 4 * SMALL + W1COL + E + K]
        bias_t_s = mf_s[:, GOFF_BIAS:GOFF_BIAS + 8]
        dmask_s = mf_s[:, GOFF_DMASK:GOFF_DMASK + NV]
        consts_s = mf_s[:, GOFF_CONSTS:GOFF_CONSTS + E + 2]

        ones_s = pool.tile([1, 128], DT)
        nc.vector.memset(ones_s[:], 1.0)

        # ---- PE warmup: dummy matmuls lift the HAM clock gate -----------
        warm_s = pool.tile([128, 512], DT)
        nc.gpsimd.memset(warm_s[:], 0.0)
        for i in range(8):
            pw = psum.tile([128, 512], F32, tag="w1")
            nc.tensor.matmul(pw[:], warm_s[:, 0:128], warm_s[:],
                             start=True, stop=True)

        def bc(ap, like):
            a, _ = broadcast_tensor_aps(ap, like)
            return a

        # ---- gather: acc[p,c,j] = qv[p,c, crx[p,c,j]] (DVE, overlaps PE) --
        acc_s = pool.tile([128, C, JW], DT)
        mask_s = pool.tile([128, C, JW], U16)
        for a in range(NA):
            nc.vector.tensor_scalar(mask_s[:], crx_s, float(a), None, OP.is_equal)
            data = qvx_s[:, :, a, :]
            if a == 0:
                nc.vector.tensor_tensor(acc_s[:], mask_s[:], data, OP.mult)
            else:
                nc.vector.copy_predicated(acc_s[:], mask_s[:], data)

        # ---- stage-1 transposed GEMMs: z = relu(Wcat.T @ states + b) ----
        z_s = pool.tile([128, 6, R], DT)      # zA | z1 | z2 (feat-major)
        zad_s = pool.tile([128, 2, 128], DT)  # diag zA, chunk 0 only
        for fc in range(6):
            for rh in range(2):
                p1 = psum.tile([128, 512], F32, tag="s1t")
                for kc in range(4):
                    nc.tensor.matmul(
                        p1[:], wcat(kc, fc * 128, (fc + 1) * 128),
                        s_t(kc, rh * 512, (rh + 1) * 512),
                        start=(kc == 0), stop=(kc == 3))
                nc.scalar.activation(z_s[:, fc, rh * 512:(rh + 1) * 512], p1[:],
                                     AF.Relu, bias=bias_t_s[:, fc:fc + 1])
                if rh == 0 and fc < 2:
                    nc.scalar.activation(zad_s[:, fc, :], p1[:, 0:128],
                                         AF.Relu, bias=bias_t_s[:, 6 + fc:7 + fc])

        # ---- stage-1 row-major small heads ------------------------------
        small_s = pool.tile([128, C, SMALL], F32)  # w01|b01|b00|b1|zb2(pre-relu)
        for g in range(2):
            p2 = psum.tile([128, 4, SMALL], F32, tag="s1r")
            nc.tensor.matmul(p2[:].rearrange("p a b -> p (a b)"),
                             ones_s[:], brow_small4_s,
                             start=True, stop=False)
            for rl in range(4):
                rc = g * 4 + rl
                for kc in range(4):
                    nc.tensor.matmul(p2[:, rl, :],
                                     s_t(kc, rc * 128, (rc + 1) * 128),
                                     wsmall(kc), start=False, stop=(kc == 3))
            nc.scalar.copy(small_s[:, g * 4:(g + 1) * 4, :], p2[:])
        zb2r_s = pool.tile([128, C, E], F32)
        nc.vector.tensor_single_scalar(zb2r_s[:], small_s[:, :, 44:76], 0.0, OP.max)

        # ---- stage-2: w2 / w0c / w0d first (small, unblock group chain) --
        w20_s = pool.tile([128, C, E + K], DT)  # |w2| cols 0:32, |w0c| 32:36
        w2r_s = w20_s[:, :, 0:E]
        w0c_s = w20_s[:, :, E:E + K]
        for rc in range(C):
            p4 = psum.tile([128, E + K], F32, tag="w20")
            nc.tensor.matmul(p4[:], ones_s[:], brow_w20_s,
                             start=True, stop=False)
            for kc in range(2):
                nc.tensor.matmul(p4[:, 0:E], z_s[:, 4 + kc, rc * 128:(rc + 1) * 128],
                                 w2l2(kc), start=False, stop=(kc == 1))
                nc.tensor.matmul(p4[:, E:E + K],
                                 z_s[:, 0 + kc, rc * 128:(rc + 1) * 128],
                                 w0l2(kc), start=False, stop=(kc == 1))
            nc.scalar.activation(w20_s[:, rc, :], p4[:], AF.Abs)
        w0d_s = pool.tile([128, K], DT)       # |w0_diag|, chunk 0
        p6 = psum.tile([128, K], F32, tag="w20")
        nc.tensor.matmul(p6[:], ones_s[:], brow_w20_s[:, E:E + K],
                         start=True, stop=False)
        for kc in range(2):
            nc.tensor.matmul(p6[:], zad_s[:, kc, :], w0l2(kc),
                             start=False, stop=(kc == 1))
        nc.scalar.activation(w0d_s[:], p6[:], AF.Abs)

        # ---- group values ------------------------------------------------
        gath4 = acc_s[:, :, 0:NVK].rearrange("p c (v k) -> p c v k", k=K)
        w04 = w0c_s.rearrange("p c (o k) -> p c o k", o=1)
        prodg_s = pool.tile([128, C, NV, K], DT)
        nc.vector.tensor_tensor(prodg_s[:], gath4, bc(w04, gath4), OP.mult)
        group_s = pool.tile([128, C, NV], F32)
        nc.vector.tensor_reduce(group_s[:], prodg_s[:], AX.X, OP.add)
        gb = small_s[:, :, 11:12]
        nc.vector.tensor_tensor(group_s[:], group_s[:], bc(gb, group_s[:]), OP.add)

        # ---- diag correction (chunk 0) ----------------------------------
        dw_s = pool.tile([128, K], F32)
        nc.vector.tensor_tensor(dw_s[:], w0d_s[:], w0c_s[:, 0, :], OP.subtract)
        gselp_s = pool.tile([128, K], F32)
        nc.vector.tensor_tensor(gselp_s[:], acc_s[:, 0, NVK:JW], dw_s[:], OP.mult)
        corr0_s = pool.tile([128, 1], F32)
        nc.vector.tensor_reduce(corr0_s[:], gselp_s[:], AX.X, OP.add)
        corr_s = pool.tile([128, 1], F32)
        nc.vector.tensor_scalar(corr_s[:], corr0_s[:],
                                consts_s[:, E:E + 1], None, OP.add)
        nc.vector.scalar_tensor_tensor(group_s[:, 0, :], dmask_s, corr_s[:],
                                       group_s[:, 0, :], OP.mult, OP.add)

        # ---- "other" residual head --------------------------------------
        prodo_s = pool.tile([128, C, NA], F32)
        nc.vector.tensor_tensor(prodo_s[:], qvb_s, small_s[:, :, 0:NA], OP.mult)
        other_s = pool.tile([128, C], F32)
        nc.vector.tensor_reduce(other_s[:], prodo_s[:], AX.X, OP.add)
        nc.vector.tensor_tensor(other_s[:], other_s[:], small_s[:, :, NA], OP.add)

        # ---- gq assembly -------------------------------------------------
        gq_s = pool.tile([128, C, NV + 1], DT)
        nc.vector.tensor_copy(gq_s[:, :, 0:NV], group_s[:])
        nc.vector.tensor_copy(gq_s[:, :, NV:NV + 1],
                              other_s[:].rearrange("p (c o) -> p c o", o=1))

        # ---- b2 head + w2 row-sums (early; only need small_s / w20_s) ---
        w2sum_s = pool.tile([128, C], F32)
        nc.vector.tensor_reduce(w2sum_s[:], w2r_s, AX.X, OP.add)
        b2p_s = pool.tile([128, C, E], F32)
        cb2 = consts_s[:, 0:E].rearrange("p (o e) -> p o e", o=1)
        nc.vector.tensor_tensor(b2p_s[:], zb2r_s[:], bc(cb2, zb2r_s[:]), OP.mult)
        b2v_s = pool.tile([128, C], F32)
        nc.vector.tensor_reduce(b2v_s[:], b2p_s[:], AX.X, OP.add)
        wsub_s = pool.tile([128, C], F32)
        nc.vector.tensor_tensor(wsub_s[:], w2sum_s[:], b2v_s[:], OP.subtract)

        # ---- stage-2 w1 GEMMs, fused per-rc with the hidden mix ---------
        # w1r cols are e-major (col = e*17+v); mix[p,rc,e] = sum_v gq*|w1|
        w1r_s = pool.tile([128, C, W1COL], DT)
        mix_s = pool.tile([128, C, E], F32)
        for rc in range(C):
            for h in range(2):
                p3 = psum.tile([128, 272], F32, tag="w1")
                nc.tensor.matmul(p3[:], ones_s[:],
                                 brow_w1_s[:, h * 272:(h + 1) * 272],
                                 start=True, stop=False)
                for kc in range(2):
                    nc.tensor.matmul(p3[:], z_s[:, 2 + kc, rc * 128:(rc + 1) * 128],
                                     w1l2(kc, h * 272, (h + 1) * 272),
                                     start=False, stop=(kc == 1))
                nc.scalar.activation(w1r_s[:, rc, h * 272:(h + 1) * 272],
                                     p3[:], AF.Abs)
            w1v = w1r_s[:, rc, :].rearrange("p (e v) -> p e v", v=NV + 1)
            gqv = gq_s[:, rc, :].rearrange("p (o v) -> p o v", o=1)
            prodh = hpool.tile([128, E, NV + 1], DT, tag="prodh")
            eng = nc.vector if rc % 2 == 0 else nc.gpsimd
            eng.tensor_tensor(prodh[:], w1v, bc(gqv, w1v), OP.mult)
            nc.vector.tensor_reduce(mix_s[:, rc, :], prodh[:], AX.X, OP.add)

        # ---- hidden = elu(mix + b1), y = sum_e (hid-1)*|w2| + b2 --------
        hidp_s = pool.tile([128, C, E], F32)
        nc.vector.tensor_tensor(hidp_s[:], mix_s[:], small_s[:, :, 12:44], OP.add)
        m_s = pool.tile([128, C, E], F32)
        nc.vector.tensor_single_scalar(m_s[:], hidp_s[:], 0.0, OP.min)
        e_s = pool.tile([128, C, E], F32)
        nc.scalar.activation(e_s[:], m_s[:], AF.Exp)
        hid_s = pool.tile([128, C, E], F32)   # = elu(hidp) + 1
        nc.vector.scalar_tensor_tensor(hid_s[:], hidp_s[:], 0.0, e_s[:],
                                       OP.max, OP.add)
        prodf_s = pool.tile([128, C, E], F32)
        nc.vector.tensor_tensor(prodf_s[:], hid_s[:], w2r_s, OP.mult)
        ysum_s = pool.tile([128, C], F32)
        nc.vector.tensor_reduce(ysum_s[:], prodf_s[:], AX.X, OP.add)
        y_s = pool.tile([128, C], F32)
        # y = (ysum + b2_l2_b) - (w2sum - b2v);  wsub precomputed early
        nc.vector.scalar_tensor_tensor(y_s[:], ysum_s[:],
                                       consts_s[:, E + 1:E + 2], wsub_s[:],
                                       OP.add, OP.subtract)
        nc.sync.dma_start(out_d[:], y_s[:])

    nc.compile()
    return nc


def _prep_inputs(inputs):
    g = lambda k: np.asarray(inputs[k], dtype=np.float32)
    states = g('states')
    qvals = g('qvals')
    cr = np.asarray(inputs['causal_relations'])

    w00_l1_W, w00_l1_b = g('w00_l1_W'), g('w00_l1_b')
    b00_W, b00_b = g('b00_W'), g('b00_b')
    h_delta = w00_l1_W[SD:].sum(0)
    g_delta = float(b00_W[SD:].sum(0)[0])

    wcat = np.concatenate([w00_l1_W[:SD], g('w1_l1_W'), g('w2_l1_W')], axis=1)
    b_cat = np.concatenate([w00_l1_b, g('w1_l1_b'), g('w2_l1_b')])
    bias_t = np.zeros((128, 8), np.float32)
    for fc in range(6):
        bias_t[:, fc] = b_cat[fc * 128:(fc + 1) * 128]
    for fc in range(2):
        bias_t[:, 6 + fc] = (w00_l1_b + h_delta)[fc * 128:(fc + 1) * 128]

    wsmall = np.concatenate([g('w01_W'), g('b01_W'), b00_W[:SD],
                             g('b1_W'), g('b2_l1_W')], axis=1)
    brow_small = np.concatenate([g('w01_b'), g('b01_b'), b00_b,
                                 g('b1_b'), g('b2_l1_b')])
    perm = np.array([v * E + e for e in range(E) for v in range(NV + 1)])
    w1l2 = g('w1_l2_W')[:, perm]
    brow_w1 = g('w1_l2_b')[perm]
    w2l2, brow_w2 = g('w2_l2_W'), g('w2_l2_b')
    w0l2, brow_w0 = g('w00_l2_W'), g('w00_l2_b')

    # shared bf16 mega columns (everything except s_t / crx / qvb)
    mb_shared = np.zeros((128, NBF), BF)
    for kc in range(4):
        mb_shared[:, OFF_PAIR + kc * PAIR:OFF_PAIR + kc * PAIR + 768] = \
            wcat[kc * 128:(kc + 1) * 128]
        mb_shared[:, OFF_WSMALL + kc * SMALL:OFF_WSMALL + (kc + 1) * SMALL] = \
            wsmall[kc * 128:(kc + 1) * 128]
    for kc in range(2):
        mb_shared[:, OFF_W1L2 + kc * W1COL:OFF_W1L2 + (kc + 1) * W1COL] = \
            w1l2[kc * 128:(kc + 1) * 128]
        mb_shared[:, OFF_W2L2 + kc * E:OFF_W2L2 + (kc + 1) * E] = \
            w2l2[kc * 128:(kc + 1) * 128]
        mb_shared[:, OFF_W0L2 + kc * K:OFF_W0L2 + (kc + 1) * K] = \
            w0l2[kc * 128:(kc + 1) * 128]
    o = OFF_BROW
    mb_shared[0, o:o + 4 * SMALL] = np.tile(brow_small, 4)
    mb_shared[0, o + 4 * SMALL:o + 4 * SMALL + W1COL] = brow_w1
    mb_shared[0, o + 4 * SMALL + W1COL:o + 4 * SMALL + W1COL + E + K] = \
        np.concatenate([brow_w2, brow_w0])

    mf_shared = np.zeros((128, NF32), np.float32)
    mf_shared[:, GOFF_BIAS:GOFF_BIAS + 8] = bias_t
    mf_shared[:, GOFF_CONSTS:GOFF_CONSTS + E] = g('b2_l2_W')[:, 0][None, :]
    mf_shared[:, GOFF_CONSTS + E] = g_delta
    mf_shared[:, GOFF_CONSTS + E + 1] = float(g('b2_l2_b')[0])

    to_pc = lambda x: np.ascontiguousarray(
        x.reshape(C, 128, -1).transpose(1, 0, 2).reshape(128, -1))

    in_maps = []
    for m in range(NCORES):
        bs = m + 8 * np.arange(16)
        mb = mb_shared.copy()
        S2 = states[bs].reshape(R, SD)
        s_tT = np.ascontiguousarray(S2.T).astype(BF)    # [512, R]
        for kc in range(4):
            mb[:, OFF_PAIR + kc * PAIR + 768:OFF_PAIR + (kc + 1) * PAIR] = \
                s_tT[kc * 128:(kc + 1) * 128]

        qv = qvals[bs].reshape(R, NA)
        cr_vk = np.swapaxes(cr[bs].reshape(R, K, NV), 1, 2)  # [r, v, k]
        crx = np.zeros((R, JW), np.float32)
        crx[:, 0:NVK] = cr_vk.reshape(R, NVK)
        vd = np.where(np.arange(128) < 64, m, m + 8)
        crx[0:128, NVK:JW] = cr_vk[np.arange(128), vd, :]
        mb[:, OFF_CRX:OFF_CRX + C * JW] = to_pc(crx)
        qv_pc = to_pc(qv)
        mb[:, OFF_QVB:OFF_QVB + C * NA] = qv_pc
        mb[:, OFF_QVX:OFF_TAIL] = np.broadcast_to(
            qv_pc.reshape(128, C, NA, 1).astype(BF),
            (128, C, NA, JW)).reshape(128, -1)

        mf = mf_shared.copy()
        dmask = np.zeros((128, NV), np.float32)
        dmask[np.arange(128), vd] = 1.0
        mf[:, GOFF_DMASK:GOFF_DMASK + NV] = dmask
        in_maps.append(dict(mb=mb, mf=mf))
    return in_maps


def kernel(**inputs):
    if 'nc' not in _cache:
        _cache['nc'] = _build_nc()
    nc = _cache['nc']
    in_maps = _prep_inputs(inputs)
    res = run_bass_kernel_spmd(nc, in_maps, list(range(NCORES)),
                               **_cache.get('run_kwargs', {}))
    _cache['last_result'] = res
    y = np.zeros((B, T, 1), np.float32)
    for m in range(NCORES):
        bs = m + 8 * np.arange(16)
        o = res.results[m]['out']               # [128, C]
        rows = np.ascontiguousarray(o.T).reshape(R)   # r = c*128+p
        y[bs] = rows.reshape(16, T, 1)
    return y


# revision 11
# speedup vs baseline: 1.2187x; 1.0297x over previous
"""Trainium2 Bass kernel for nn_CausalMixer (QMIX-style causal mixer).

Data-parallel across 8 NeuronCores: batch dim sharded round-robin
(core m gets batches m, m+8, m+16, ...), hypernet weights replicated.

Per-core layout (R = 1024 rows = 16 batches x 64 timesteps):
  - stage-1 "transposed" GEMMs: out[feat, rows] = Wcat.T-chunks @ states.T,
    evicted with fused per-partition bias+ReLU on ScalarE.
  - stage-2 row-major GEMMs: the relu'd z tiles [feat, rows] serve directly
    as lhsT, producing per-row hypernet weights [rows, feat]; bias is
    preloaded into PSUM with a K=1 ones-matmul.
  - gather (qvals[cr]) via 10x {is_equal mask + copy_predicated} on DVE.
  - the onehot quirk (batch row b==v gets +delta) is handled as a rank-1
    correction on chunk 0 only (host orders the diag batches first).
  - all bf16 inputs ride one mega-packed DRAM tensor (few big DMAs,
    need-ordered columns); dummy matmuls at t=0 lift the PE HAM throttle.
"""

import sys

for _p in ("/root/.axon_site", "/root/.axon_site/_ro/trn_rl_repo",
           "/root/.axon_site/_ro/pypackages", "/opt/trn_rl_repo"):
    if _p not in sys.path:
        sys.path.append(_p)

import numpy as np
import ml_dtypes
from contextlib import ExitStack

import concourse.bass as bass
import concourse.bacc as bacc_mod
import concourse.tile as tile
import concourse.mybir as mybir
from concourse.bass import broadcast_tensor_aps
from concourse.bass_utils import run_bass_kernel_spmd

BF = ml_dtypes.bfloat16
DT = mybir.dt.bfloat16
F32 = mybir.dt.float32
U16 = mybir.dt.uint16
OP = mybir.AluOpType
AF = mybir.ActivationFunctionType
AX = mybir.AxisListType

NCORES = 8
B, T, NA, NV, K, SD, H, E = 128, 64, 10, 16, 4, 512, 256, 32
R = 16 * T            # rows per core = 1024
C = R // 128          # row chunks per core = 8
NVK = NV * K          # 64
JW = NVK + K          # 68 gather cols (64 + 4 diag)
SMALL = NA + 1 + 1 + E + E   # 76: w01 | b01 | b00 | b1 | b2_l1
W1COL = (NV + 1) * E  # 544

# ---- mega-packed bf16 input column map (need-ordered) -------------------
# crx 8x68 | qvb 8x10, then 4x [wcat_kc (768) | s_t_kc (1024)] pairs,
# then qvx 8x10x68, then tail:
#   wsmall 4x76 | w1l2 2x544 | w2l2 2x32 | w0l2 2x4
#   | brow_small4 4x76 | brow_w1 544 | brow_w20 36  (partition 0)
PAIR = 768 + R
OFF_CRX = 0
OFF_QVB = OFF_CRX + C * JW
OFF_PAIR = OFF_QVB + C * NA
OFF_QVX = OFF_PAIR + 4 * PAIR
OFF_TAIL = OFF_QVX + C * NA * JW
OFF_WSMALL = OFF_TAIL
OFF_W1L2 = OFF_WSMALL + 4 * SMALL
OFF_W2L2 = OFF_W1L2 + 2 * W1COL
OFF_W0L2 = OFF_W2L2 + 2 * E
OFF_BROW = OFF_W0L2 + 2 * K
NBF = OFF_BROW + 4 * SMALL + W1COL + E + K
# f32 mega: bias_t (8) | dmask (16) | consts (34)
GOFF_BIAS = 0
GOFF_DMASK = 8
GOFF_CONSTS = 24
NF32 = GOFF_CONSTS + E + 2

_cache = {}


def _build_nc():
    nc = bacc_mod.Bacc("TRN2", target_bir_lowering=False, debug=False)

    mb_d = nc.dram_tensor("mb", [128, NBF], DT, kind="ExternalInput")
    mf_d = nc.dram_tensor("mf", [128, NF32], F32, kind="ExternalInput")
    out_d = nc.dram_tensor("out", [128, C], F32, kind="ExternalOutput")

    with tile.TileContext(nc) as tc, ExitStack() as ctx:
        pool = ctx.enter_context(tc.tile_pool(name="sbuf", bufs=1))
        hpool = ctx.enter_context(tc.tile_pool(name="hbuf", bufs=3))
        psum = ctx.enter_context(tc.tile_pool(name="psum", bufs=2, space="PSUM"))

        mb_s = pool.tile([128, NBF], DT)
        mf_s = pool.tile([128, NF32], F32)
        # need-ordered loads: crx/qvb, tail (weights+bias rows), f32 consts,
        # then the big (wcat,s_t) kc pairs, qvx last
        nc.sync.dma_start(mb_s[:, OFF_CRX:OFF_PAIR], mb_d[:, OFF_CRX:OFF_PAIR])
        nc.sync.dma_start(mb_s[:, OFF_TAIL:NBF], mb_d[:, OFF_TAIL:NBF])
        nc.sync.dma_start(mf_s[:], mf_d[:])
        for kc in range(4):
            nc.sync.dma_start(
                mb_s[:, OFF_PAIR + kc * PAIR:OFF_PAIR + (kc + 1) * PAIR],
                mb_d[:, OFF_PAIR + kc * PAIR:OFF_PAIR + (kc + 1) * PAIR])
        nc.sync.dma_start(mb_s[:, OFF_QVX:OFF_TAIL], mb_d[:, OFF_QVX:OFF_TAIL])

        def wcat(kc, c0, c1):
            return mb_s[:, OFF_PAIR + kc * PAIR + c0:OFF_PAIR + kc * PAIR + c1]

        def s_t(kc, c0, c1):
            return mb_s[:, OFF_PAIR + kc * PAIR + 768 + c0:
                        OFF_PAIR + kc * PAIR + 768 + c1]

        def wsmall(kc):
            return mb_s[:, OFF_WSMALL + kc * SMALL:OFF_WSMALL + (kc + 1) * SMALL]

        def w1l2(kc, c0, c1):
            return mb_s[:, OFF_W1L2 + kc * W1COL + c0:OFF_W1L2 + kc * W1COL + c1]

        def w2l2(kc):
            return mb_s[:, OFF_W2L2 + kc * E:OFF_W2L2 + (kc + 1) * E]

        def w0l2(kc):
            return mb_s[:, OFF_W0L2 + kc * K:OFF_W0L2 + (kc + 1) * K]

        crx_s = mb_s[:, OFF_CRX:OFF_CRX + C * JW].rearrange(
            "p (c j) -> p c j", j=JW)
        qvb_s = mb_s[:, OFF_QVB:OFF_QVB + C * NA].rearrange(
            "p (c j) -> p c j", j=NA)
        qvx_s = mb_s[:, OFF_QVX:OFF_TAIL].rearrange(
            "p (c a j) -> p c a j", a=NA, j=JW)
        brow_small4_s = mb_s[0:1, OFF_BROW:OFF_BROW + 4 * SMALL]
        brow_w1_s = mb_s[0:1, OFF_BROW + 4 * SMALL:OFF_BROW + 4 * SMALL + W1COL]
        brow_w20_s = mb_s[0:1, OFF_BROW + 4 * SMALL + W1COL:
                          OFF_BROW + 4 * SMALL + W1COL + E + K]
        bias_t_s = mf_s[:, GOFF_BIAS:GOFF_BIAS + 8]
        dmask_s = mf_s[:, GOFF_DMASK:GOFF_DMASK + NV]
        consts_s = mf_s[:, GOFF_CONSTS:GOFF_CONSTS + E + 2]

        ones_s = pool.tile([1, 128], DT)
        nc.vector.memset(ones_s[:], 1.0)

        # ---- PE warmup: dummy matmuls lift the HAM clock gate -----------
        warm_s = pool.tile([128, 512], DT)
        nc.vector.memset(warm_s[:], 0.0)
        for i in range(16):
            pw = psum.tile([128, 512], F32, tag="w1")
            nc.tensor.matmul(pw[:], warm_s[:, 0:128], warm_s[:],
                             start=True, stop=True)

        def bc(ap, like):
            a, _ = broadcast_tensor_aps(ap, like)
            return a

        # ---- gather: acc[p,c,j] = qv[p,c, crx[p,c,j]] (DVE, overlaps PE) --
        acc_s = pool.tile([128, C, JW], DT)
        mask_s = pool.tile([128, C, JW], U16)
        for a in range(NA):
            nc.vector.tensor_scalar(mask_s[:], crx_s, float(a), None, OP.is_equal)
            data = qvx_s[:, :, a, :]
            if a == 0:
                nc.vector.tensor_tensor(acc_s[:], mask_s[:], data, OP.mult)
            else:
                nc.vector.copy_predicated(acc_s[:], mask_s[:], data)

        # ---- stage-1 transposed GEMMs: z = relu(Wcat.T @ states + b) ----
        z_s = pool.tile([128, 6, R], DT)      # zA | z1 | z2 (feat-major)
        zad_s = pool.tile([128, 2, 128], DT)  # diag zA, chunk 0 only
        for fc in (0, 1, 4, 5, 2, 3):
            for rh in range(2):
                p1 = psum.tile([128, 512], F32, tag="s1t")
                for kc in range(4):
                    nc.tensor.matmul(
                        p1[:], wcat(kc, fc * 128, (fc + 1) * 128),
                        s_t(kc, rh * 512, (rh + 1) * 512),
                        start=(kc == 0), stop=(kc == 3))
                nc.scalar.activation(z_s[:, fc, rh * 512:(rh + 1) * 512], p1[:],
                                     AF.Relu, bias=bias_t_s[:, fc:fc + 1])
                if rh == 0 and fc < 2:
                    nc.scalar.activation(zad_s[:, fc, :], p1[:, 0:128],
                                         AF.Relu, bias=bias_t_s[:, 6 + fc:7 + fc])

        # ---- stage-1 row-major small heads ------------------------------
        small_s = pool.tile([128, C, SMALL], F32)  # w01|b01|b00|b1|zb2(pre-relu)
        for g in range(2):
            p2 = psum.tile([128, 4, SMALL], F32, tag="s1r")
            nc.tensor.matmul(p2[:].rearrange("p a b -> p (a b)"),
                             ones_s[:], brow_small4_s,
                             start=True, stop=False)
            for rl in range(4):
                rc = g * 4 + rl
                for kc in range(4):
                    nc.tensor.matmul(p2[:, rl, :],
                                     s_t(kc, rc * 128, (rc + 1) * 128),
                                     wsmall(kc), start=False, stop=(kc == 3))
            nc.scalar.copy(small_s[:, g * 4:(g + 1) * 4, :], p2[:])
        zb2r_s = pool.tile([128, C, E], F32)
        nc.vector.tensor_single_scalar(zb2r_s[:], small_s[:, :, 44:76], 0.0, OP.max)

        # ---- stage-2: w2 / w0c / w0d first (small, unblock group chain) --
        w20_s = pool.tile([128, C, E + K], DT)  # |w2| cols 0:32, |w0c| 32:36
        w2r_s = w20_s[:, :, 0:E]
        w0c_s = w20_s[:, :, E:E + K]
        for rc in range(C):
            p4 = psum.tile([128, E + K], F32, tag="w20")
            nc.tensor.matmul(p4[:], ones_s[:], brow_w20_s,
                             start=True, stop=False)
            for kc in range(2):
                nc.tensor.matmul(p4[:, 0:E], z_s[:, 4 + kc, rc * 128:(rc + 1) * 128],
                                 w2l2(kc), start=False, stop=(kc == 1))
                nc.tensor.matmul(p4[:, E:E + K],
                                 z_s[:, 0 + kc, rc * 128:(rc + 1) * 128],
                                 w0l2(kc), start=False, stop=(kc == 1))
            nc.scalar.activation(w20_s[:, rc, :], p4[:], AF.Abs)
        w0d_s = pool.tile([128, K], DT)       # |w0_diag|, chunk 0
        p6 = psum.tile([128, K], F32, tag="w20")
        nc.tensor.matmul(p6[:], ones_s[:], brow_w20_s[:, E:E + K],
                         start=True, stop=False)
        for kc in range(2):
            nc.tensor.matmul(p6[:], zad_s[:, kc, :], w0l2(kc),
                             start=False, stop=(kc == 1))
        nc.scalar.activation(w0d_s[:], p6[:], AF.Abs)

        # ---- group values ------------------------------------------------
        gath4 = acc_s[:, :, 0:NVK].rearrange("p c (v k) -> p c v k", k=K)
        w04 = w0c_s.rearrange("p c (o k) -> p c o k", o=1)
        prodg_s = pool.tile([128, C, NV, K], DT)
        nc.vector.tensor_tensor(prodg_s[:], gath4, bc(w04, gath4), OP.mult)
        group_s = pool.tile([128, C, NV], F32)
        nc.vector.tensor_reduce(group_s[:], prodg_s[:], AX.X, OP.add)
        gb = small_s[:, :, 11:12]
        nc.vector.tensor_tensor(group_s[:], group_s[:], bc(gb, group_s[:]), OP.add)

        # ---- diag correction (chunk 0) ----------------------------------
        dw_s = pool.tile([128, K], F32)
        nc.vector.tensor_tensor(dw_s[:], w0d_s[:], w0c_s[:, 0, :], OP.subtract)
        gselp_s = pool.tile([128, K], F32)
        nc.vector.tensor_tensor(gselp_s[:], acc_s[:, 0, NVK:JW], dw_s[:], OP.mult)
        corr0_s = pool.tile([128, 1], F32)
        nc.vector.tensor_reduce(corr0_s[:], gselp_s[:], AX.X, OP.add)
        corr_s = pool.tile([128, 1], F32)
        nc.vector.tensor_scalar(corr_s[:], corr0_s[:],
                                consts_s[:, E:E + 1], None, OP.add)
        nc.vector.scalar_tensor_tensor(group_s[:, 0, :], dmask_s, corr_s[:],
                                       group_s[:, 0, :], OP.mult, OP.add)

        # ---- "other" residual head --------------------------------------
        prodo_s = pool.tile([128, C, NA], F32)
        nc.vector.tensor_tensor(prodo_s[:], qvb_s, small_s[:, :, 0:NA], OP.mult)
        other_s = pool.tile([128, C], F32)
        nc.vector.tensor_reduce(other_s[:], prodo_s[:], AX.X, OP.add)
        nc.vector.tensor_tensor(other_s[:], other_s[:], small_s[:, :, NA], OP.add)

        # ---- gq assembly -------------------------------------------------
        gq_s = pool.tile([128, C, NV + 1], DT)
        nc.vector.tensor_copy(gq_s[:, :, 0:NV], group_s[:])
        nc.vector.tensor_copy(gq_s[:, :, NV:NV + 1],
                              other_s[:].rearrange("p (c o) -> p c o", o=1))

        # ---- b2 head + w2 row-sums (early; only need small_s / w20_s) ---
        w2sum_s = pool.tile([128, C], F32)
        nc.vector.tensor_reduce(w2sum_s[:], w2r_s, AX.X, OP.add)
        b2p_s = pool.tile([128, C, E], F32)
        cb2 = consts_s[:, 0:E].rearrange("p (o e) -> p o e", o=1)
        nc.vector.tensor_tensor(b2p_s[:], zb2r_s[:], bc(cb2, zb2r_s[:]), OP.mult)
        b2v_s = pool.tile([128, C], F32)
        nc.vector.tensor_reduce(b2v_s[:], b2p_s[:], AX.X, OP.add)
        wsub_s = pool.tile([128, C], F32)
        nc.vector.tensor_tensor(wsub_s[:], w2sum_s[:], b2v_s[:], OP.subtract)

        # ---- stage-2 w1 GEMMs, fused per-rc with the hidden mix ---------
        # w1r cols are e-major (col = e*17+v); mix[p,rc,e] = sum_v gq*|w1|
        w1r_s = pool.tile([128, C, W1COL], DT)
        mix_s = pool.tile([128, C, E], F32)
        for rc in range(C):
            for h in range(2):
                p3 = psum.tile([128, 272], F32, tag="w1")
                nc.tensor.matmul(p3[:], ones_s[:],
                                 brow_w1_s[:, h * 272:(h + 1) * 272],
                                 start=True, stop=False)
                for kc in range(2):
                    nc.tensor.matmul(p3[:], z_s[:, 2 + kc, rc * 128:(rc + 1) * 128],
                                     w1l2(kc, h * 272, (h + 1) * 272),
                                     start=False, stop=(kc == 1))
                nc.scalar.activation(w1r_s[:, rc, h * 272:(h + 1) * 272],
                                     p3[:], AF.Abs)
            w1v = w1r_s[:, rc, :].rearrange("p (e v) -> p e v", v=NV + 1)
            gqv = gq_s[:, rc, :].rearrange("p (o v) -> p o v", o=1)
            prodh = hpool.tile([128, E, NV + 1], DT, tag="prodh")
            eng = nc.vector if rc % 2 == 0 else nc.gpsimd
            eng.tensor_tensor(prodh[:], w1v, bc(gqv, w1v), OP.mult)
            nc.vector.tensor_reduce(mix_s[:, rc, :], prodh[:], AX.X, OP.add)

        # ---- hidden = elu(mix + b1), y = sum_e (hid-1)*|w2| + b2 --------
        hidp_s = pool.tile([128, C, E], F32)
        nc.vector.tensor_tensor(hidp_s[:], mix_s[:], small_s[:, :, 12:44], OP.add)
        m_s = pool.tile([128, C, E], F32)
        nc.vector.tensor_single_scalar(m_s[:], hidp_s[:], 0.0, OP.min)
        e_s = pool.tile([128, C, E], F32)
        nc.scalar.activation(e_s[:], m_s[:], AF.Exp)
        hid_s = pool.tile([128, C, E], F32)   # = elu(hidp) + 1
        nc.vector.scalar_tensor_tensor(hid_s[:], hidp_s[:], 0.0, e_s[:],
                                       OP.max, OP.add)
        prodf_s = pool.tile([128, C, E], F32)
        nc.vector.tensor_tensor(prodf_s[:], hid_s[:], w2r_s, OP.mult)
        ysum_s = pool.tile([128, C], F32)
        nc.vector.tensor_reduce(ysum_s[:], prodf_s[:], AX.X, OP.add)
        y_s = pool.tile([128, C], F32)
        # y = (ysum + b2_l2_b) - (w2sum - b2v);  wsub precomputed early
        nc.vector.scalar_tensor_tensor(y_s[:], ysum_s[:],
                                       consts_s[:, E + 1:E + 2], wsub_s[:],
                                       OP.add, OP.subtract)
        nc.sync.dma_start(out_d[:], y_s[:])

    nc.compile()
    return nc


def _prep_inputs(inputs):
    g = lambda k: np.asarray(inputs[k], dtype=np.float32)
    states = g('states')
    qvals = g('qvals')
    cr = np.asarray(inputs['causal_relations'])

    w00_l1_W, w00_l1_b = g('w00_l1_W'), g('w00_l1_b')
    b00_W, b00_b = g('b00_W'), g('b00_b')
    h_delta = w00_l1_W[SD:].sum(0)
    g_delta = float(b00_W[SD:].sum(0)[0])

    wcat = np.concatenate([w00_l1_W[:SD], g('w1_l1_W'), g('w2_l1_W')], axis=1)
    b_cat = np.concatenate([w00_l1_b, g('w1_l1_b'), g('w2_l1_b')])
    bias_t = np.zeros((128, 8), np.float32)
    for fc in range(6):
        bias_t[:, fc] = b_cat[fc * 128:(fc + 1) * 128]
    for fc in range(2):
        bias_t[:, 6 + fc] = (w00_l1_b + h_delta)[fc * 128:(fc + 1) * 128]

    wsmall = np.concatenate([g('w01_W'), g('b01_W'), b00_W[:SD],
                             g('b1_W'), g('b2_l1_W')], axis=1)
    brow_small = np.concatenate([g('w01_b'), g('b01_b'), b00_b,
                                 g('b1_b'), g('b2_l1_b')])
    perm = np.array([v * E + e for e in range(E) for v in range(NV + 1)])
    w1l2 = g('w1_l2_W')[:, perm]
    brow_w1 = g('w1_l2_b')[perm]
    w2l2, brow_w2 = g('w2_l2_W'), g('w2_l2_b')
    w0l2, brow_w0 = g('w00_l2_W'), g('w00_l2_b')

    # shared bf16 mega columns (everything except s_t / crx / qvb)
    mb_shared = np.zeros((128, NBF), BF)
    for kc in range(4):
        mb_shared[:, OFF_PAIR + kc * PAIR:OFF_PAIR + kc * PAIR + 768] = \
            wcat[kc * 128:(kc + 1) * 128]
        mb_shared[:, OFF_WSMALL + kc * SMALL:OFF_WSMALL + (kc + 1) * SMALL] = \
            wsmall[kc * 128:(kc + 1) * 128]
    for kc in range(2):
        mb_shared[:, OFF_W1L2 + kc * W1COL:OFF_W1L2 + (kc + 1) * W1COL] = \
            w1l2[kc * 128:(kc + 1) * 128]
        mb_shared[:, OFF_W2L2 + kc * E:OFF_W2L2 + (kc + 1) * E] = \
            w2l2[kc * 128:(kc + 1) * 128]
        mb_shared[:, OFF_W0L2 + kc * K:OFF_W0L2 + (kc + 1) * K] = \
            w0l2[kc * 128:(kc + 1) * 128]
    o = OFF_BROW
    mb_shared[0, o:o + 4 * SMALL] = np.tile(brow_small, 4)
    mb_shared[0, o + 4 * SMALL:o + 4 * SMALL + W1COL] = brow_w1
    mb_shared[0, o + 4 * SMALL + W1COL:o + 4 * SMALL + W1COL + E + K] = \
        np.concatenate([brow_w2, brow_w0])

    mf_shared = np.zeros((128, NF32), np.float32)
    mf_shared[:, GOFF_BIAS:GOFF_BIAS + 8] = bias_t
    mf_shared[:, GOFF_CONSTS:GOFF_CONSTS + E] = g('b2_l2_W')[:, 0][None, :]
    mf_shared[:, GOFF_CONSTS + E] = g_delta
    mf_shared[:, GOFF_CONSTS + E + 1] = float(g('b2_l2_b')[0])

    to_pc = lambda x: np.ascontiguousarray(
        x.reshape(C, 128, -1).transpose(1, 0, 2).reshape(128, -1))

    in_maps = []
    for m in range(NCORES):
        bs = m + 8 * np.arange(16)
        mb = mb_shared.copy()
        S2 = states[bs].reshape(R, SD)
        s_tT = np.ascontiguousarray(S2.T).astype(BF)    # [512, R]
        for kc in range(4):
            mb[:, OFF_PAIR + kc * PAIR + 768:OFF_PAIR + (kc + 1) * PAIR] = \
                s_tT[kc * 128:(kc + 1) * 128]

        qv = qvals[bs].reshape(R, NA)
        cr_vk = np.swapaxes(cr[bs].reshape(R, K, NV), 1, 2)  # [r, v, k]
        crx = np.zeros((R, JW), np.float32)
        crx[:, 0:NVK] = cr_vk.reshape(R, NVK)
        vd = np.where(np.arange(128) < 64, m, m + 8)
        crx[0:128, NVK:JW] = cr_vk[np.arange(128), vd, :]
        mb[:, OFF_CRX:OFF_CRX + C * JW] = to_pc(crx)
        qv_pc = to_pc(qv)
        mb[:, OFF_QVB:OFF_QVB + C * NA] = qv_pc
        mb[:, OFF_QVX:OFF_TAIL] = np.broadcast_to(
            qv_pc.reshape(128, C, NA, 1).astype(BF),
            (128, C, NA, JW)).reshape(128, -1)

        mf = mf_shared.copy()
        dmask = np.zeros((128, NV), np.float32)
        dmask[np.arange(128), vd] = 1.0
        mf[:, GOFF_DMASK:GOFF_DMASK + NV] = dmask
        in_maps.append(dict(mb=mb, mf=mf))
    return in_maps


def kernel(**inputs):
    if 'nc' not in _cache:
        _cache['nc'] = _build_nc()
    nc = _cache['nc']
    in_maps = _prep_inputs(inputs)
    res = run_bass_kernel_spmd(nc, in_maps, list(range(NCORES)),
                               **_cache.get('run_kwargs', {}))
    _cache['last_result'] = res
    y = np.zeros((B, T, 1), np.float32)
    for m in range(NCORES):
        bs = m + 8 * np.arange(16)
        o = res.results[m]['out']               # [128, C]
        rows = np.ascontiguousarray(o.T).reshape(R)   # r = c*128+p
        y[bs] = rows.reshape(16, T, 1)
    return y


# revision 12
# speedup vs baseline: 1.3633x; 1.1187x over previous
"""Trainium2 Bass kernel for nn_CausalMixer (QMIX-style causal mixer).

Data-parallel across 8 NeuronCores: batch dim sharded round-robin
(core m gets batches m, m+8, m+16, ...), hypernet weights replicated.

Per-core layout (R = 1024 rows = 16 batches x 64 timesteps):
  - stage-1 "transposed" GEMMs: out[feat, rows] = Wcat.T-chunks @ states.T,
    evicted with fused per-partition bias+ReLU on ScalarE.
  - stage-2 row-major GEMMs: the relu'd z tiles [feat, rows] serve directly
    as lhsT, producing per-row hypernet weights [rows, feat]; bias is
    preloaded into PSUM with a K=1 ones-matmul.
  - gather (qvals[cr]) via 10x {is_equal mask + copy_predicated} on DVE.
  - the onehot quirk (batch row b==v gets +delta) is handled as a rank-1
    correction on chunk 0 only (host orders the diag batches first).
  - all bf16 inputs ride one mega-packed DRAM tensor (few big DMAs,
    need-ordered columns); dummy matmuls at t=0 lift the PE HAM throttle.
"""

import sys

for _p in ("/root/.axon_site", "/root/.axon_site/_ro/trn_rl_repo",
           "/root/.axon_site/_ro/pypackages", "/opt/trn_rl_repo"):
    if _p not in sys.path:
        sys.path.append(_p)

import numpy as np
import ml_dtypes
from contextlib import ExitStack

import concourse.bass as bass
import concourse.bacc as bacc_mod
import concourse.tile as tile
import concourse.mybir as mybir
from concourse.bass import broadcast_tensor_aps
from concourse.bass_utils import run_bass_kernel_spmd

BF = ml_dtypes.bfloat16
DT = mybir.dt.bfloat16
F32 = mybir.dt.float32
U16 = mybir.dt.uint16
OP = mybir.AluOpType
AF = mybir.ActivationFunctionType
AX = mybir.AxisListType

NCORES = 8
B, T, NA, NV, K, SD, H, E = 128, 64, 10, 16, 4, 512, 256, 32
R = 16 * T            # rows per core = 1024
C = R // 128          # row chunks per core = 8
NVK = NV * K          # 64
JW = NVK + K          # 68 gather cols (64 + 4 diag)
SMALL = NA + 1 + 1 + E + E   # 76: w01 | b01 | b00 | b1 | b2_l1
W1COL = (NV + 1) * E  # 544

# ---- mega-packed bf16 input column map (need-ordered) -------------------
# crx 8x68 | qvb 8x10, then 4x [wcat_kc (768) | s_t_kc (1024)] pairs,
# then qvx 8x10x68, then tail:
#   wsmall 4x76 | w1l2 2x544 | w2l2 2x32 | w0l2 2x4
#   | brow_small4 4x76 | brow_w1 544 | brow_w20 36  (partition 0)
PAIR = 768 + R
OFF_CRX = 0
OFF_QVB = OFF_CRX + C * JW
OFF_PAIR = OFF_QVB + C * NA
OFF_TAIL = OFF_PAIR + 4 * PAIR
OFF_WSMALL = OFF_TAIL
OFF_W1L2 = OFF_WSMALL + 4 * SMALL
OFF_W2L2 = OFF_W1L2 + 2 * W1COL
OFF_W0L2 = OFF_W2L2 + 2 * E
OFF_BROW = OFF_W0L2 + 2 * K
NBF = OFF_BROW + 4 * SMALL + W1COL + E + K
# f32 mega: bias_t (8) | dmask (16) | consts (34)
GOFF_BIAS = 0
GOFF_DMASK = 8
GOFF_CONSTS = 24
NF32 = GOFF_CONSTS + E + 2

_cache = {}


def _build_nc():
    nc = bacc_mod.Bacc("TRN2", target_bir_lowering=False, debug=False)

    mb_d = nc.dram_tensor("mb", [128, NBF], DT, kind="ExternalInput")
    mf_d = nc.dram_tensor("mf", [128, NF32], F32, kind="ExternalInput")
    out_d = nc.dram_tensor("out", [128, C], F32, kind="ExternalOutput")

    with tile.TileContext(nc) as tc, ExitStack() as ctx:
        pool = ctx.enter_context(tc.tile_pool(name="sbuf", bufs=1))
        hpool = ctx.enter_context(tc.tile_pool(name="hbuf", bufs=3))
        psum = ctx.enter_context(tc.tile_pool(name="psum", bufs=2, space="PSUM"))

        mb_s = pool.tile([128, NBF], DT)
        mf_s = pool.tile([128, NF32], F32)
        # need-ordered loads: crx/qvb, tail (weights+bias rows), f32 consts,
        # then the big (wcat,s_t) kc pairs, qvx last
        nc.sync.dma_start(mb_s[:, OFF_CRX:OFF_PAIR], mb_d[:, OFF_CRX:OFF_PAIR])
        nc.sync.dma_start(mb_s[:, OFF_TAIL:NBF], mb_d[:, OFF_TAIL:NBF])
        nc.sync.dma_start(mf_s[:], mf_d[:])
        for kc in range(4):
            nc.sync.dma_start(
                mb_s[:, OFF_PAIR + kc * PAIR:OFF_PAIR + (kc + 1) * PAIR],
                mb_d[:, OFF_PAIR + kc * PAIR:OFF_PAIR + (kc + 1) * PAIR])

        def wcat(kc, c0, c1):
            return mb_s[:, OFF_PAIR + kc * PAIR + c0:OFF_PAIR + kc * PAIR + c1]

        def s_t(kc, c0, c1):
            return mb_s[:, OFF_PAIR + kc * PAIR + 768 + c0:
                        OFF_PAIR + kc * PAIR + 768 + c1]

        def wsmall(kc):
            return mb_s[:, OFF_WSMALL + kc * SMALL:OFF_WSMALL + (kc + 1) * SMALL]

        def w1l2(kc, c0, c1):
            return mb_s[:, OFF_W1L2 + kc * W1COL + c0:OFF_W1L2 + kc * W1COL + c1]

        def w2l2(kc):
            return mb_s[:, OFF_W2L2 + kc * E:OFF_W2L2 + (kc + 1) * E]

        def w0l2(kc):
            return mb_s[:, OFF_W0L2 + kc * K:OFF_W0L2 + (kc + 1) * K]

        crx_s = mb_s[:, OFF_CRX:OFF_CRX + C * JW].rearrange(
            "p (c j) -> p c j", j=JW)
        qvb_s = mb_s[:, OFF_QVB:OFF_QVB + C * NA].rearrange(
            "p (c j) -> p c j", j=NA)
        brow_small4_s = mb_s[0:1, OFF_BROW:OFF_BROW + 4 * SMALL]
        brow_w1_s = mb_s[0:1, OFF_BROW + 4 * SMALL:OFF_BROW + 4 * SMALL + W1COL]
        brow_w20_s = mb_s[0:1, OFF_BROW + 4 * SMALL + W1COL:
                          OFF_BROW + 4 * SMALL + W1COL + E + K]
        bias_t_s = mf_s[:, GOFF_BIAS:GOFF_BIAS + 8]
        dmask_s = mf_s[:, GOFF_DMASK:GOFF_DMASK + NV]
        consts_s = mf_s[:, GOFF_CONSTS:GOFF_CONSTS + E + 2]

        ones_s = pool.tile([1, 128], DT)
        nc.vector.memset(ones_s[:], 1.0)

        # ---- PE warmup: dummy matmuls lift the HAM clock gate -----------
        warm_s = pool.tile([128, 512], DT)
        nc.vector.memset(warm_s[:], 0.0)
        for i in range(10):
            pw = psum.tile([128, 512], F32, tag="w1", bufs=3)
            nc.tensor.matmul(pw[:], warm_s[:, 0:128], warm_s[:],
                             start=True, stop=True)

        def bc(ap, like):
            a, _ = broadcast_tensor_aps(ap, like)
            return a

        # ---- gather: acc[p,c,j] = qv[p,c, crx[p,c,j]] (DVE, overlaps PE) --
        acc_s = pool.tile([128, C, JW], DT)
        mask_s = pool.tile([128, C, JW], U16)
        for a in range(NA):
            nc.vector.tensor_scalar(mask_s[:], crx_s, float(a), None, OP.is_equal)
            data = bc(qvb_s[:, :, a:a + 1], mask_s[:])
            if a == 0:
                nc.vector.tensor_tensor(acc_s[:], mask_s[:], data, OP.mult)
            else:
                nc.vector.copy_predicated(acc_s[:], mask_s[:], data)

        # ---- stage-1 transposed GEMMs: z = relu(Wcat.T @ states + b) ----
        z_s = pool.tile([128, 6, R], DT)      # zA | z1 | z2 (feat-major)
        zad_s = pool.tile([128, 2, 128], DT)  # diag zA, chunk 0 only

        def s1t_pass(fcs):
            for fc in fcs:
                for rh in range(2):
                    p1 = psum.tile([128, 512], F32, tag="s1t", bufs=2,
                                   name="p1")
                    for kc in range(4):
                        nc.tensor.matmul(
                            p1[:], wcat(kc, fc * 128, (fc + 1) * 128),
                            s_t(kc, rh * 512, (rh + 1) * 512),
                            start=(kc == 0), stop=(kc == 3))
                    nc.scalar.activation(z_s[:, fc, rh * 512:(rh + 1) * 512],
                                         p1[:], AF.Relu,
                                         bias=bias_t_s[:, fc:fc + 1])
                    if rh == 0 and fc < 2:
                        nc.scalar.activation(zad_s[:, fc, :], p1[:, 0:128],
                                             AF.Relu,
                                             bias=bias_t_s[:, 6 + fc:7 + fc])

        s1t_pass((0, 1, 4, 5))

        # ---- stage-1 row-major small heads ------------------------------
        small_s = pool.tile([128, C, SMALL], F32)  # w01|b01|b00|b1|zb2(pre-relu)
        for g in range(2):
            p2 = psum.tile([128, 4, SMALL], F32, tag="s1r", bufs=1)
            nc.tensor.matmul(p2[:].rearrange("p a b -> p (a b)"),
                             ones_s[:], brow_small4_s,
                             start=True, stop=False)
            for rl in range(4):
                rc = g * 4 + rl
                for kc in range(4):
                    nc.tensor.matmul(p2[:, rl, :],
                                     s_t(kc, rc * 128, (rc + 1) * 128),
                                     wsmall(kc), start=False, stop=(kc == 3))
            nc.scalar.copy(small_s[:, g * 4:(g + 1) * 4, :], p2[:])
        zb2r_s = pool.tile([128, C, E], F32)
        nc.vector.tensor_single_scalar(zb2r_s[:], small_s[:, :, 44:76], 0.0, OP.max)

        # ---- stage-2: w2 / w0c / w0d first (small, unblock group chain) --
        w20_s = pool.tile([128, C, E + K], DT)  # |w2| cols 0:32, |w0c| 32:36
        w2r_s = w20_s[:, :, 0:E]
        w0c_s = w20_s[:, :, E:E + K]
        for rc in range(C):
            p4 = psum.tile([128, E + K], F32, tag="w20", bufs=2)
            nc.tensor.matmul(p4[:], ones_s[:], brow_w20_s,
                             start=True, stop=False)
            for kc in range(2):
                nc.tensor.matmul(p4[:, 0:E], z_s[:, 4 + kc, rc * 128:(rc + 1) * 128],
                                 w2l2(kc), start=False, stop=(kc == 1))
                nc.tensor.matmul(p4[:, E:E + K],
                                 z_s[:, 0 + kc, rc * 128:(rc + 1) * 128],
                                 w0l2(kc), start=False, stop=(kc == 1))
            nc.scalar.activation(w20_s[:, rc, :], p4[:], AF.Abs)
        w0d_s = pool.tile([128, K], DT)       # |w0_diag|, chunk 0
        p6 = psum.tile([128, K], F32, tag="w20", bufs=2)
        nc.tensor.matmul(p6[:], ones_s[:], brow_w20_s[:, E:E + K],
                         start=True, stop=False)
        for kc in range(2):
            nc.tensor.matmul(p6[:], zad_s[:, kc, :], w0l2(kc),
                             start=False, stop=(kc == 1))
        nc.scalar.activation(w0d_s[:], p6[:], AF.Abs)

        # ---- group values ------------------------------------------------
        gath4 = acc_s[:, :, 0:NVK].rearrange("p c (v k) -> p c v k", k=K)
        w04 = w0c_s.rearrange("p c (o k) -> p c o k", o=1)
        prodg_s = pool.tile([128, C, NV, K], DT)
        nc.vector.tensor_tensor(prodg_s[:], gath4, bc(w04, gath4), OP.mult)
        group_s = pool.tile([128, C, NV], F32)
        nc.vector.tensor_reduce(group_s[:], prodg_s[:], AX.X, OP.add)
        gb = small_s[:, :, 11:12]
        nc.vector.tensor_tensor(group_s[:], group_s[:], bc(gb, group_s[:]), OP.add)

        # ---- diag correction (chunk 0) ----------------------------------
        dw_s = pool.tile([128, K], F32)
        nc.vector.tensor_tensor(dw_s[:], w0d_s[:], w0c_s[:, 0, :], OP.subtract)
        gselp_s = pool.tile([128, K], F32)
        nc.vector.tensor_tensor(gselp_s[:], acc_s[:, 0, NVK:JW], dw_s[:], OP.mult)
        corr0_s = pool.tile([128, 1], F32)
        nc.vector.tensor_reduce(corr0_s[:], gselp_s[:], AX.X, OP.add)
        corr_s = pool.tile([128, 1], F32)
        nc.vector.tensor_scalar(corr_s[:], corr0_s[:],
                                consts_s[:, E:E + 1], None, OP.add)
        nc.vector.scalar_tensor_tensor(group_s[:, 0, :], dmask_s, corr_s[:],
                                       group_s[:, 0, :], OP.mult, OP.add)

        # ---- "other" residual head --------------------------------------
        prodo_s = pool.tile([128, C, NA], F32)
        nc.vector.tensor_tensor(prodo_s[:], qvb_s, small_s[:, :, 0:NA], OP.mult)
        other_s = pool.tile([128, C], F32)
        nc.vector.tensor_reduce(other_s[:], prodo_s[:], AX.X, OP.add)
        nc.vector.tensor_tensor(other_s[:], other_s[:], small_s[:, :, NA], OP.add)

        # ---- gq assembly -------------------------------------------------
        gq_s = pool.tile([128, C, NV + 1], DT)
        nc.vector.tensor_copy(gq_s[:, :, 0:NV], group_s[:])
        nc.vector.tensor_copy(gq_s[:, :, NV:NV + 1],
                              other_s[:].rearrange("p (c o) -> p c o", o=1))

        s1t_pass((2, 3))

        # ---- b2 head + w2 row-sums (early; only need small_s / w20_s) ---
        w2sum_s = pool.tile([128, C], F32)
        nc.vector.tensor_reduce(w2sum_s[:], w2r_s, AX.X, OP.add)
        b2p_s = pool.tile([128, C, E], F32)
        cb2 = consts_s[:, 0:E].rearrange("p (o e) -> p o e", o=1)
        nc.vector.tensor_tensor(b2p_s[:], zb2r_s[:], bc(cb2, zb2r_s[:]), OP.mult)
        b2v_s = pool.tile([128, C], F32)
        nc.vector.tensor_reduce(b2v_s[:], b2p_s[:], AX.X, OP.add)
        wsub_s = pool.tile([128, C], F32)
        nc.vector.tensor_tensor(wsub_s[:], w2sum_s[:], b2v_s[:], OP.subtract)

        # ---- stage-2 w1 GEMMs, fused per-rc with the hidden mix ---------
        # w1r cols are e-major (col = e*17+v); mix[p,rc,e] = sum_v gq*|w1|
        w1r_s = pool.tile([128, C, W1COL], DT)
        mix_s = pool.tile([128, C, E], F32)
        for rc in range(C):
            for h in range(2):
                p3 = psum.tile([128, 272], F32, tag="w1", bufs=3)
                nc.tensor.matmul(p3[:], ones_s[:],
                                 brow_w1_s[:, h * 272:(h + 1) * 272],
                                 start=True, stop=False)
                for kc in range(2):
                    nc.tensor.matmul(p3[:], z_s[:, 2 + kc, rc * 128:(rc + 1) * 128],
                                     w1l2(kc, h * 272, (h + 1) * 272),
                                     start=False, stop=(kc == 1))
                nc.scalar.activation(w1r_s[:, rc, h * 272:(h + 1) * 272],
                                     p3[:], AF.Abs)
            w1v = w1r_s[:, rc, :].rearrange("p (e v) -> p e v", v=NV + 1)
            gqv = gq_s[:, rc, :].rearrange("p (o v) -> p o v", o=1)
            prodh = hpool.tile([128, E, NV + 1], DT, tag="prodh")
            eng = nc.vector if rc % 2 == 0 else nc.gpsimd
            eng.tensor_tensor(prodh[:], w1v, bc(gqv, w1v), OP.mult)
            nc.vector.tensor_reduce(mix_s[:, rc, :], prodh[:], AX.X, OP.add)

        # ---- hidden = elu(mix + b1), y = sum_e (hid-1)*|w2| + b2 --------
        hidp_s = pool.tile([128, C, E], F32)
        nc.vector.tensor_tensor(hidp_s[:], mix_s[:], small_s[:, :, 12:44], OP.add)
        m_s = pool.tile([128, C, E], F32)
        nc.vector.tensor_single_scalar(m_s[:], hidp_s[:], 0.0, OP.min)
        e_s = pool.tile([128, C, E], F32)
        nc.scalar.activation(e_s[:], m_s[:], AF.Exp)
        hid_s = pool.tile([128, C, E], F32)   # = elu(hidp) + 1
        nc.vector.scalar_tensor_tensor(hid_s[:], hidp_s[:], 0.0, e_s[:],
                                       OP.max, OP.add)
        prodf_s = pool.tile([128, C, E], F32)
        nc.vector.tensor_tensor(prodf_s[:], hid_s[:], w2r_s, OP.mult)
        ysum_s = pool.tile([128, C], F32)
        nc.vector.tensor_reduce(ysum_s[:], prodf_s[:], AX.X, OP.add)
        y_s = pool.tile([128, C], F32)
        # y = (ysum + b2_l2_b) - (w2sum - b2v);  wsub precomputed early
        nc.vector.scalar_tensor_tensor(y_s[:], ysum_s[:],
                                       consts_s[:, E + 1:E + 2], wsub_s[:],
                                       OP.add, OP.subtract)
        nc.sync.dma_start(out_d[:], y_s[:])

    nc.compile()
    return nc


def _prep_inputs(inputs):
    g = lambda k: np.asarray(inputs[k], dtype=np.float32)
    states = g('states')
    qvals = g('qvals')
    cr = np.asarray(inputs['causal_relations'])

    w00_l1_W, w00_l1_b = g('w00_l1_W'), g('w00_l1_b')
    b00_W, b00_b = g('b00_W'), g('b00_b')
    h_delta = w00_l1_W[SD:].sum(0)
    g_delta = float(b00_W[SD:].sum(0)[0])

    wcat = np.concatenate([w00_l1_W[:SD], g('w1_l1_W'), g('w2_l1_W')], axis=1)
    b_cat = np.concatenate([w00_l1_b, g('w1_l1_b'), g('w2_l1_b')])
    bias_t = np.zeros((128, 8), np.float32)
    for fc in range(6):
        bias_t[:, fc] = b_cat[fc * 128:(fc + 1) * 128]
    for fc in range(2):
        bias_t[:, 6 + fc] = (w00_l1_b + h_delta)[fc * 128:(fc + 1) * 128]

    wsmall = np.concatenate([g('w01_W'), g('b01_W'), b00_W[:SD],
                             g('b1_W'), g('b2_l1_W')], axis=1)
    brow_small = np.concatenate([g('w01_b'), g('b01_b'), b00_b,
                                 g('b1_b'), g('b2_l1_b')])
    perm = np.array([v * E + e for e in range(E) for v in range(NV + 1)])
    w1l2 = g('w1_l2_W')[:, perm]
    brow_w1 = g('w1_l2_b')[perm]
    w2l2, brow_w2 = g('w2_l2_W'), g('w2_l2_b')
    w0l2, brow_w0 = g('w00_l2_W'), g('w00_l2_b')

    # shared bf16 mega columns (everything except s_t / crx / qvb)
    mb_shared = np.zeros((128, NBF), BF)
    for kc in range(4):
        mb_shared[:, OFF_PAIR + kc * PAIR:OFF_PAIR + kc * PAIR + 768] = \
            wcat[kc * 128:(kc + 1) * 128]
        mb_shared[:, OFF_WSMALL + kc * SMALL:OFF_WSMALL + (kc + 1) * SMALL] = \
            wsmall[kc * 128:(kc + 1) * 128]
    for kc in range(2):
        mb_shared[:, OFF_W1L2 + kc * W1COL:OFF_W1L2 + (kc + 1) * W1COL] = \
            w1l2[kc * 128:(kc + 1) * 128]
        mb_shared[:, OFF_W2L2 + kc * E:OFF_W2L2 + (kc + 1) * E] = \
            w2l2[kc * 128:(kc + 1) * 128]
        mb_shared[:, OFF_W0L2 + kc * K:OFF_W0L2 + (kc + 1) * K] = \
            w0l2[kc * 128:(kc + 1) * 128]
    o = OFF_BROW
    mb_shared[0, o:o + 4 * SMALL] = np.tile(brow_small, 4)
    mb_shared[0, o + 4 * SMALL:o + 4 * SMALL + W1COL] = brow_w1
    mb_shared[0, o + 4 * SMALL + W1COL:o + 4 * SMALL + W1COL + E + K] = \
        np.concatenate([brow_w2, brow_w0])

    mf_shared = np.zeros((128, NF32), np.float32)
    mf_shared[:, GOFF_BIAS:GOFF_BIAS + 8] = bias_t
    mf_shared[:, GOFF_CONSTS:GOFF_CONSTS + E] = g('b2_l2_W')[:, 0][None, :]
    mf_shared[:, GOFF_CONSTS + E] = g_delta
    mf_shared[:, GOFF_CONSTS + E + 1] = float(g('b2_l2_b')[0])

    to_pc = lambda x: np.ascontiguousarray(
        x.reshape(C, 128, -1).transpose(1, 0, 2).reshape(128, -1))

    in_maps = []
    for m in range(NCORES):
        bs = m + 8 * np.arange(16)
        mb = mb_shared.copy()
        S2 = states[bs].reshape(R, SD)
        s_tT = np.ascontiguousarray(S2.T).astype(BF)    # [512, R]
        for kc in range(4):
            mb[:, OFF_PAIR + kc * PAIR + 768:OFF_PAIR + (kc + 1) * PAIR] = \
                s_tT[kc * 128:(kc + 1) * 128]

        qv = qvals[bs].reshape(R, NA)
        cr_vk = np.swapaxes(cr[bs].reshape(R, K, NV), 1, 2)  # [r, v, k]
        crx = np.zeros((R, JW), np.float32)
        crx[:, 0:NVK] = cr_vk.reshape(R, NVK)
        vd = np.where(np.arange(128) < 64, m, m + 8)
        crx[0:128, NVK:JW] = cr_vk[np.arange(128), vd, :]
        mb[:, OFF_CRX:OFF_CRX + C * JW] = to_pc(crx)
        mb[:, OFF_QVB:OFF_QVB + C * NA] = to_pc(qv)

        mf = mf_shared.copy()
        dmask = np.zeros((128, NV), np.float32)
        dmask[np.arange(128), vd] = 1.0
        mf[:, GOFF_DMASK:GOFF_DMASK + NV] = dmask
        in_maps.append(dict(mb=mb, mf=mf))
    return in_maps


def kernel(**inputs):
    if 'nc' not in _cache:
        _cache['nc'] = _build_nc()
    nc = _cache['nc']
    in_maps = _prep_inputs(inputs)
    res = run_bass_kernel_spmd(nc, in_maps, list(range(NCORES)),
                               **_cache.get('run_kwargs', {}))
    _cache['last_result'] = res
    y = np.zeros((B, T, 1), np.float32)
    for m in range(NCORES):
        bs = m + 8 * np.arange(16)
        o = res.results[m]['out']               # [128, C]
        rows = np.ascontiguousarray(o.T).reshape(R)   # r = c*128+p
        y[bs] = rows.reshape(16, T, 1)
    return y
